# revision 7
# baseline (speedup 1.0000x reference)
"""MeshGraphNet forward on 8 Trainium2 NeuronCores (Bass/Tile).

Sharding: nodes in 8 contiguous blocks (batch is sorted, so graph segments
stay contiguous); edges colocated with their receiver node, grouped by
128-node block and padded so every block owns exactly T_B 128-edge tiles.
With edges partitioned by receiver, all message passing is core-local:
scatter-mean is a per-block one-hot matmul (1/deg folded into the one-hot);
the only cross-core exchange is a [128,4] AllReduce for global mean-pooling.

Layouts: activations are feature-major [128 feat, cols] for matmuls;
LayerNorm stats/apply detour through edge-major via PE transposes.
LN beta vectors are folded into downstream consumer biases on the host
(deg-0 receivers get a rank-1 correction matmul), so the device only
applies gamma. Matmuls run as float32r (full PE rate at N>=256); the
edge-latent residual state is stored bf16.
"""

import sys

for _p in ("/opt/trn_rl_repo",):
    if _p not in sys.path:
        sys.path.insert(0, _p)

from contextlib import ExitStack

import ml_dtypes
import numpy as np

import concourse.bass as bass
import concourse.mybir as mybir
import concourse.tile as tile
from concourse.bass_utils import run_bass_kernel_spmd
from concourse.masks import make_identity

P = 128
NC = 8
H = 128
NG = 4
OUT = 3
L = 4
LN_EPS = 1e-5

f32 = mybir.dt.float32
f32r = mybir.dt.float32r
bf16 = mybir.dt.bfloat16
AF = mybir.ActivationFunctionType
ALU = mybir.AluOpType
BF = ml_dtypes.bfloat16


def _np(x):
    return np.asarray(x, dtype=np.float32)


def split_excess_waits(nc, maxw=1):
    """Walrus here rejects >1 semaphore wait per instruction; hoist extra
    waits onto preceding NoOps on the same engine."""
    nsplit = 0
    for f in nc.m.functions:
        for blk in f.blocks:
            idx = 0
            insts = blk.instructions
            while idx < len(insts):
                ins = insts[idx]
                si = ins.sync_info
                if si is None or len(si.on_wait) <= maxw:
                    idx += 1
                    continue
                waits = list(si.on_wait)
                extra, keep = waits[:-maxw], waits[-maxw:]
                while extra:
                    chunk, extra = extra[:maxw], extra[maxw:]
                    nop = mybir.InstNoOp(
                        name=nc.get_next_instruction_name(), ins=[], outs=[])
                    nop.engine = ins.engine
                    nop.sync_info = mybir.SyncInfo(on_wait=chunk, on_update=[])
                    nc.register_instruction(nop)
                    insts.insert(idx, nop)
                    idx += 1
                    nsplit += 1
                si.on_wait = keep
                ins.sync_info = si
                idx += 1
    return nsplit


# ---------------------------------------------------------------------------
# host-side preprocessing
# ---------------------------------------------------------------------------

def prep(node_attr, edge_attr, edge_index, batch, params):
    node_attr = np.asarray(node_attr, np.float32)
    edge_attr = np.asarray(edge_attr, np.float32)
    edge_index = np.asarray(edge_index, np.int64)
    batch = np.asarray(batch, np.int64)

    N, NODE_IN = node_attr.shape
    E, EDGE_IN = edge_attr.shape
    assert N % NC == 0
    npc = N // NC
    NB = (npc + P - 1) // P
    NPAD = NB * P

    col = edge_index[1]
    deg = np.bincount(col, minlength=N).astype(np.float32)
    invdeg = (1.0 / np.maximum(deg, 1.0)).astype(np.float32)

    core = col // npc
    lid = col - core * npc
    blk = lid // P
    gb = core * NB + blk
    cnt = np.bincount(gb, minlength=NC * NB)
    TB = int(np.ceil(max(1, cnt.max()) / P))
    EPC = NB * TB * P

    order = np.argsort(gb, kind="stable")
    sgb = gb[order]
    block_starts = np.searchsorted(sgb, np.arange(NC * NB))
    pos = np.arange(E) - block_starts[sgb]
    slot = sgb * TB * P + pos

    eattr_pad = np.zeros((NC * EPC, EDGE_IN), np.float32)
    eattr_pad[slot] = edge_attr[order]
    eattr = np.ascontiguousarray(
        eattr_pad.reshape(NC, EPC, EDGE_IN).transpose(0, 2, 1))

    S = np.zeros((NC * EPC, P), np.float32)
    S[slot, lid[order] % P] = invdeg[col[order]]
    S = np.ascontiguousarray(S.reshape(NC, NB * TB, P, P).astype(BF))

    nattrT = np.zeros((NC, NODE_IN, NPAD), np.float32)
    oh_fm = np.zeros((NC, NG, NPAD), np.float32)
    oh_em = np.zeros((NC, NPAD, NG), np.float32)
    deg0 = np.zeros((NC, 1, NPAD), np.float32)
    for c in range(NC):
        nattrT[c, :, :npc] = node_attr[c * npc:(c + 1) * npc].T
        bc = batch[c * npc:(c + 1) * npc]
        oh = (bc[:, None] == np.arange(NG)[None, :]).astype(np.float32)
        oh_fm[c, :, :npc] = oh.T
        oh_em[c, :npc, :] = oh
        deg0[c, 0, :npc] = (deg[c * npc:(c + 1) * npc] == 0.0)

    gcnt = np.bincount(batch, minlength=NG).astype(np.float32)
    invc = (1.0 / np.maximum(gcnt, 1.0)).astype(np.float32)[:, None]

    # ---- weights with beta folding -------------------------------------
    def WB(p):
        return [_np(w) for w in p["W"]], [_np(b) for b in p["b"]]

    pg = params
    gW, gb_ = WB(pg["glob_mlp"])
    Wl, bl = _np(pg["glob_lin"][0]), _np(pg["glob_lin"][1])
    gW4 = gW[3] @ Wl
    gb4 = gb_[3] @ Wl + bl
    glob = (gW[:3] + [gW4], gb_[:3] + [gb4])

    neW, neB = WB(pg["node_enc"])
    ne_g, ne_beta = _np(pg["node_enc"]["g"]), _np(pg["node_enc"]["beta"])
    eeW, eeB = WB(pg["edge_enc"])
    ee_g, ee_beta = _np(pg["edge_enc"]["g"]), _np(pg["edge_enc"]["beta"])

    layers = []
    Ce = ee_beta.copy()          # constant folded out of e-state
    Cx = ne_beta.copy()          # constant folded out of x-state
    for lp in pg["layers"]:
        eW, eB = WB(lp["edge"])
        e_g, e_beta = _np(lp["edge"]["g"]), _np(lp["edge"]["beta"])
        nW, nB = WB(lp["node"])
        n_g, n_beta = _np(lp["node"]["g"]), _np(lp["node"]["beta"])
        eB0 = eB[0] + Ce @ eW[0]
        Ce = Ce + e_beta
        Wx, Wa = nW[0][:H], nW[0][H:]
        nB0 = nB[0] + Cx @ Wx + Ce @ Wa
        corr = -(Ce @ Wa)
        Cx = Cx + n_beta
        layers.append(dict(eW=eW, eB=[eB0] + eB[1:], eg=e_g,
                           Wx=Wx, Wa=Wa, nW=nW, nB=[nB0] + nB[1:], ng=n_g,
                           corr=corr))

    dW, dB = WB(pg["dec"])
    dB0 = dB[0] + Cx @ dW[0]
    dec = (dW, [dB0] + dB[1:])

    return dict(
        N=N, E=E, npc=npc, NB=NB, NPAD=NPAD, TB=TB, EPC=EPC,
        NODE_IN=NODE_IN, EDGE_IN=EDGE_IN,
        eattr=eattr, S=S, nattrT=nattrT, oh_fm=oh_fm, oh_em=oh_em,
        deg0=deg0, invc=invc,
        glob=glob, node_enc=(neW, neB, ne_g), edge_enc=(eeW, eeB, ee_g),
        layers=layers, dec=dec,
    )


# ---------------------------------------------------------------------------
# weight blob packing
# ---------------------------------------------------------------------------

class Blob:
    """Concatenate [K,<=128] float32 slabs into one [R,128] DRAM tensor."""

    def __init__(self):
        self.rows = []
        self.r = 0
        self.index = {}

    def add(self, name, arr):
        arr = np.asarray(arr, np.float32)
        if arr.ndim == 1:
            arr = arr[None, :]
        k, m = arr.shape
        pad = np.zeros((k, P), np.float32)
        pad[:, :m] = arr
        self.index[name] = (self.r, k, m)
        self.rows.append(pad)
        self.r += k

    def data(self):
        return (np.concatenate(self.rows, 0) if self.rows
                else np.zeros((1, P), np.float32))


def pack_blobs(pp):
    wf = Blob()   # float32r matmul weights (lhsT layout [K, dout])
    vf = Blob()   # float32 per-feature vectors (biases, gammas)

    gW, gB = pp["glob"]
    for i, w in enumerate(gW):
        wf.add(f"glob_w{i}", w)
        vf.add(f"glob_b{i}", gB[i])
    eeW, eeB, eeg = pp["edge_enc"]
    for i, w in enumerate(eeW):
        wf.add(f"eenc_w{i}", w)
        vf.add(f"eenc_b{i}", eeB[i])
    vf.add("eenc_g", eeg)
    neW, neB, neg = pp["node_enc"]
    wf.add("nenc_wna", neW[0][:pp["NODE_IN"]])
    wf.add("nenc_wgf", neW[0][pp["NODE_IN"]:])
    for i in range(1, 4):
        wf.add(f"nenc_w{i}", neW[i])
    for i in range(4):
        vf.add(f"nenc_b{i}", neB[i])
    vf.add("nenc_g", neg)
    wb_rows = []
    for li, lp in enumerate(pp["layers"]):
        wb_rows.append(np.asarray(lp["eW"][0], np.float32).astype(BF))
        for i in range(1, 4):
            wf.add(f"l{li}_ew{i}", lp["eW"][i])
        for i in range(4):
            vf.add(f"l{li}_eb{i}", lp["eB"][i])
        vf.add(f"l{li}_eg", lp["eg"])
        wf.add(f"l{li}_wx", lp["Wx"])
        wf.add(f"l{li}_wa", lp["Wa"])
        wf.add(f"l{li}_corr", lp["corr"])
        for i in range(1, 4):
            wf.add(f"l{li}_nw{i}", lp["nW"][i])
        for i in range(4):
            vf.add(f"l{li}_nb{i}", lp["nB"][i])
        vf.add(f"l{li}_ng", lp["ng"])
    dW, dB = pp["dec"]
    for i, w in enumerate(dW):
        wf.add(f"dec_w{i}", w)
        vf.add(f"dec_b{i}", dB[i])
    wb = np.concatenate(wb_rows, 0)  # [4*128, 128] bf16 (layer edge W1)
    return wf, vf, wb


# ---------------------------------------------------------------------------
# device kernel
# ---------------------------------------------------------------------------

def build(pp, wf, vf):
    NPAD, NB, TB, EPC = pp["NPAD"], pp["NB"], pp["TB"], pp["EPC"]
    NODE_IN, EDGE_IN = pp["NODE_IN"], pp["EDGE_IN"]

    def grps(total):
        out, c = [], 0
        while c < total:
            n = min(512, total - c)
            out.append((c, n))
            c += n
        return out

    egrp = grps(EPC)
    ngrp = grps(NPAD)

    nc = bass.Bass()
    d_wf = nc.declare_dram_parameter("wf", list(wf.data().shape), f32r, isOutput=False)
    d_vf = nc.declare_dram_parameter("vf", list(vf.data().shape), f32, isOutput=False)
    d_wb = nc.declare_dram_parameter("wb", [L * H, H], bf16, isOutput=False)
    d_ea = nc.declare_dram_parameter("eattr", [EDGE_IN, EPC], f32r, isOutput=False)
    d_na = nc.declare_dram_parameter("nattr", [NODE_IN, NPAD], f32r, isOutput=False)
    d_S = nc.declare_dram_parameter("S", [NB * TB, P, P], bf16, isOutput=False)
    d_ofm = nc.declare_dram_parameter("oh_fm", [NG, NPAD], f32r, isOutput=False)
    d_oem = nc.declare_dram_parameter("oh_em", [NPAD, NG], f32r, isOutput=False)
    d_d0 = nc.declare_dram_parameter("deg0", [1, NPAD], f32r, isOutput=False)
    d_ic = nc.declare_dram_parameter("invc", [NG, 1], f32, isOutput=False)
    d_out = nc.declare_dram_parameter("out", [OUT, NPAD], f32, isOutput=True)

    cc_in = nc.dram_tensor("cc_in", [P, NG], f32)
    cc_out = nc.dram_tensor("cc_out", [P, NG], f32)

    ctx = ExitStack()
    with ctx:
        tc = ctx.enter_context(tile.TileContext(nc))
        singles = ctx.enter_context(tc.tile_pool(name="singles", bufs=1))
        epool = ctx.enter_context(tc.tile_pool(name="e", bufs=1))
        xpool = ctx.enter_context(tc.tile_pool(name="x", bufs=1))
        apool = ctx.enter_context(tc.tile_pool(name="agg", bufs=1))
        inp = ctx.enter_context(tc.tile_pool(name="inp", bufs=3))
        hwork = ctx.enter_context(tc.tile_pool(name="hwork", bufs=3))
        stat = ctx.enter_context(tc.tile_pool(name="stat", bufs=4))
        spool = ctx.enter_context(tc.tile_pool(name="S", bufs=2))
        mmps = ctx.enter_context(tc.tile_pool(name="mmps", bufs=3, space="PSUM"))
        emps = ctx.enter_context(tc.tile_pool(name="emps", bufs=2, space="PSUM"))
        smps = ctx.enter_context(tc.tile_pool(name="smps", bufs=2, space="PSUM"))

        # ---- constants and weights ------------------------------------
        ident_f = singles.tile([P, P], f32)
        make_identity(nc, ident_f[:])
        ident_r = singles.tile([P, P], f32r)
        nc.vector.tensor_copy(out=ident_r[:], in_=ident_f[:])
        ident_b = singles.tile([P, P], bf16)
        nc.vector.tensor_copy(out=ident_b[:], in_=ident_f[:])
        eps_t = singles.tile([P, 1], f32)
        nc.vector.memset(eps_t[:], LN_EPS)

        WT = {}
        for name, (r, k, m) in wf.index.items():
            t = singles.tile([k, P], f32r, tag=f"wf_{name}", name=f"wf_{name}")
            nc.sync.dma_start(out=t[:], in_=d_wf[r:r + k, :])
            WT[name] = t
        VT = {}
        for name, (r, k, m) in vf.index.items():
            t = singles.tile([P, 1], f32, tag=f"vf_{name}", name=f"vf_{name}")
            nc.sync.dma_start(
                out=t[:m, :], in_=d_vf[r:r + 1, :m].rearrange("o k -> k o"))
            VT[name] = t
        WB1 = []
        for li in range(L):
            t = singles.tile([H, H], bf16, tag=f"wb_{li}", name=f"wb_{li}")
            nc.sync.dma_start(out=t[:], in_=d_wb[li * H:(li + 1) * H, :])
            WB1.append(t)

        oem_sb = singles.tile([P, NB, NG], f32r)
        nc.sync.dma_start(out=oem_sb[:],
                          in_=d_oem[:].rearrange("(b p) g -> p b g", p=P))
        ic_sb = singles.tile([NG, 1], f32)
        nc.sync.dma_start(out=ic_sb[:], in_=d_ic[:])

        e_sb = [epool.tile([P, n], bf16, tag=f"e{i}", name=f"e{i}")
                for i, (c, n) in enumerate(egrp)]
        x_sb = [xpool.tile([P, n], f32r, tag=f"x{i}", name=f"x{i}")
                for i, (c, n) in enumerate(ngrp)]
        a_sb = [apool.tile([P, n], f32r, tag=f"a{i}", name=f"a{i}")
                for i, (c, n) in enumerate(ngrp)]

        # ---- helpers ---------------------------------------------------
        relu_eng = ["act", "dve", "act"]

        def epilogue(dst, ps_ap, bias, relu, eng):
            if eng == "act" or not relu:
                nc.scalar.activation(out=dst, in_=ps_ap,
                                     func=AF.Relu if relu else AF.Identity,
                                     bias=bias[:])
            else:
                nc.vector.tensor_scalar(
                    out=dst, in0=ps_ap, scalar1=bias[:], scalar2=0.0,
                    op0=ALU.add, op1=ALU.max)

        def mm_chain(n, first_mms, Ws, Bs):
            """first_mms: [(lhsT_ap, rhs_ap)] accumulated; then ReLU+linear
            per W in Ws (bias Bs). Returns final psum tile [P,512]."""
            ps = mmps.tile([P, 512], f32, tag="mm")
            for j, (lh, rh) in enumerate(first_mms):
                nc.tensor.matmul(out=ps[:, :n], lhsT=lh, rhs=rh,
                                 start=(j == 0), stop=(j == len(first_mms) - 1))
            cur = ps
            for i, W in enumerate(Ws):
                h = hwork.tile([P, 512], f32r, tag="h")
                epilogue(h[:, :n], cur[:, :n], Bs[i], True, relu_eng[i % 3])
                ps2 = mmps.tile([P, 512], f32, tag="mm")
                nc.tensor.matmul(out=ps2[:, :n], lhsT=W[:], rhs=h[:, :n],
                                 start=True, stop=True)
                cur = ps2
            return cur

        def ln_apply(ps_h4, n, b4, g, resid, dst):
            """dst <- LayerNorm(ps_h4 + b4) * g (+ resid)."""
            nt = n // P
            h4b = hwork.tile([P, 512], f32r, tag="h4b")
            nc.scalar.activation(out=h4b[:, :n], in_=ps_h4[:, :n],
                                 func=AF.Identity, bias=b4[:])
            em = emps.tile([P, 512], f32r, tag="em")
            for i in range(nt):
                nc.tensor.transpose(out=em[:, i * P:(i + 1) * P],
                                    in_=h4b[:, i * P:(i + 1) * P],
                                    identity=ident_r[:])
            mv = stat.tile([P, 4, 2], f32, tag="mv")
            for i in range(nt):
                st = stat.tile([P, 6], f32, tag="st")
                nc.vector.bn_stats(out=st[:], in_=em[:, i * P:(i + 1) * P])
                nc.vector.bn_aggr(out=mv[:, i, :], in_=st[:])
            rstd = stat.tile([P, 4], f32, tag="rstd")
            murs = stat.tile([P, 4], f32, tag="murs")
            nc.scalar.activation(out=rstd[:, :nt], in_=mv[:, :nt, 1],
                                 func=AF.Sqrt, bias=eps_t[:])
            nc.vector.reciprocal(out=rstd[:, :nt], in_=rstd[:, :nt])
            nc.vector.tensor_tensor(out=murs[:, :nt], in0=mv[:, :nt, 0],
                                    in1=rstd[:, :nt], op=ALU.mult)
            tem = hwork.tile([P, 512], f32r, tag="tem")
            for i in range(nt):
                nc.vector.tensor_scalar(
                    out=tem[:, i * P:(i + 1) * P], in0=em[:, i * P:(i + 1) * P],
                    scalar1=rstd[:, i:i + 1], scalar2=murs[:, i:i + 1],
                    op0=ALU.mult, op1=ALU.subtract)
            tfm = mmps.tile([P, 512], f32r, tag="mm")
            for i in range(nt):
                nc.tensor.transpose(out=tfm[:, i * P:(i + 1) * P],
                                    in_=tem[:, i * P:(i + 1) * P],
                                    identity=ident_r[:])
            if resid is None:
                nc.vector.tensor_scalar(out=dst, in0=tfm[:, :n],
                                        scalar1=g[:], scalar2=None,
                                        op0=ALU.mult, op1=ALU.bypass)
            else:
                nc.vector.scalar_tensor_tensor(
                    out=dst, in0=tfm[:, :n], scalar=g[:], in1=resid,
                    op0=ALU.mult, op1=ALU.add)

        def load_inp(dram, c, n, kdim):
            t = inp.tile([kdim, 512], f32r, tag="inany", name="inany")
            nc.sync.dma_start(out=t[:, :n], in_=dram[:, c:c + n])
            return t

        # ---- global encoder -------------------------------------------
        partial = smps.tile([P, P], f32, tag="sm")
        for gi, (c, n) in enumerate(ngrp):
            na_t = load_inp(d_na, c, n, NODE_IN)
            ps = mm_chain(n, [(WT["glob_w0"][:], na_t[:, :n])],
                          [WT[f"glob_w{i}"] for i in range(1, 4)],
                          [VT[f"glob_b{i}"] for i in range(3)])
            xg = hwork.tile([P, 512], f32r, tag="h4b")
            nc.scalar.activation(out=xg[:, :n], in_=ps[:, :n],
                                 func=AF.Identity, bias=VT["glob_b3"][:])
            for i in range(n // P):
                b = (c // P) + i
                em = emps.tile([P, 512], f32r, tag="em")
                nc.tensor.transpose(out=em[:, :P], in_=xg[:, i * P:(i + 1) * P],
                                    identity=ident_r[:])
                emsb = hwork.tile([P, P], f32r, tag="emsb")
                nc.vector.tensor_copy(out=emsb[:], in_=em[:, :P])
                nc.tensor.matmul(out=partial[:, :NG], lhsT=emsb[:],
                                 rhs=oem_sb[:, b, :],
                                 start=(b == 0), stop=(b == NB - 1))
        partial_sb = stat.tile([P, NG], f32, tag="psb")
        nc.vector.tensor_copy(out=partial_sb[:], in_=partial[:, :NG])
        nc.sync.dma_start(out=cc_in[:], in_=partial_sb[:])
        nc.gpsimd.collective_compute(
            "AllReduce", ALU.add, ins=[cc_in[:]], outs=[cc_out[:]],
            replica_groups=[list(range(NC))])
        pooled = stat.tile([P, NG], f32r, tag="pool")
        nc.sync.dma_start(out=pooled[:], in_=cc_out[:].bitcast(f32r))
        pooledT_ps = smps.tile([P, P], f32r, tag="sm")
        nc.tensor.transpose(out=pooledT_ps[:NG, :], in_=pooled[:],
                            identity=ident_r[:])
        pooledT = stat.tile([NG, P], f32r, tag="poolT")
        nc.vector.tensor_scalar(out=pooledT[:], in0=pooledT_ps[:NG, :],
                                scalar1=ic_sb[:], scalar2=None,
                                op0=ALU.mult, op1=ALU.bypass)

        # ---- edge encoder ---------------------------------------------
        for gi, (c, n) in enumerate(egrp):
            ea_t = load_inp(d_ea, c, n, EDGE_IN)
            ps = mm_chain(n, [(WT["eenc_w0"][:], ea_t[:, :n])],
                          [WT[f"eenc_w{i}"] for i in range(1, 4)],
                          [VT[f"eenc_b{i}"] for i in range(3)])
            ln_apply(ps, n, VT["eenc_b3"], VT["eenc_g"], None, e_sb[gi][:, :n])

        # ---- node encoder ---------------------------------------------
        for gi, (c, n) in enumerate(ngrp):
            ofm_t = load_inp(d_ofm, c, n, NG)
            gf_ps = mmps.tile([P, 512], f32, tag="mm")
            nc.tensor.matmul(out=gf_ps[:, :n], lhsT=pooledT[:],
                             rhs=ofm_t[:, :n], start=True, stop=True)
            gf = hwork.tile([P, 512], f32r, tag="h4b", name="gf")
            nc.vector.tensor_copy(out=gf[:, :n], in_=gf_ps[:, :n])
            na_t = load_inp(d_na, c, n, NODE_IN)
            ps = mm_chain(
                n,
                [(WT["nenc_wna"][:], na_t[:, :n]),
                 (WT["nenc_wgf"][:], gf[:, :n])],
                [WT[f"nenc_w{i}"] for i in range(1, 4)],
                [VT[f"nenc_b{i}"] for i in range(3)])
            ln_apply(ps, n, VT["nenc_b3"], VT["nenc_g"], None, x_sb[gi][:, :n])

        # ---- message passing layers -----------------------------------
        for li in range(L):
            eB = [VT[f"l{li}_eb{i}"] for i in range(4)]
            eW = [WT[f"l{li}_ew{i}"] for i in range(1, 4)]
            for gi, (c, n) in enumerate(egrp):
                ps = mm_chain(n, [(WB1[li][:], e_sb[gi][:, :n])],
                              eW, eB[:3])
                ln_apply(ps, n, eB[3], VT[f"l{li}_eg"],
                         e_sb[gi][:, :n], e_sb[gi][:, :n])
            for b in range(NB):
                s_t = spool.tile([P, TB, P], bf16, tag="S")
                nc.sync.dma_start(
                    out=s_t[:],
                    in_=d_S[b * TB:(b + 1) * TB].rearrange("t p n -> p t n"))
                aps = smps.tile([P, P], f32, tag="sm")
                for j in range(TB):
                    t = b * TB + j
                    gi, off = (t * P) // 512, (t * P) % 512
                    em = emps.tile([P, 512], bf16, tag="em")
                    nc.tensor.transpose(out=em[:, :P],
                                        in_=e_sb[gi][:, off:off + P],
                                        identity=ident_b[:])
                    emsb = hwork.tile([P, P], bf16, tag="emsb")
                    nc.vector.tensor_copy(out=emsb[:], in_=em[:, :P])
                    nc.tensor.matmul(out=aps[:], lhsT=emsb[:],
                                     rhs=s_t[:, j, :],
                                     start=(j == 0), stop=(j == TB - 1))
                gi, off = (b * P) // 512, (b * P) % 512
                nc.vector.tensor_copy(out=a_sb[gi][:, off:off + P], in_=aps[:])
            nB = [VT[f"l{li}_nb{i}"] for i in range(4)]
            for gi, (c, n) in enumerate(ngrp):
                d0_t = load_inp(d_d0, c, n, 1)
                ps = mm_chain(
                    n,
                    [(WT[f"l{li}_wx"][:], x_sb[gi][:, :n]),
                     (WT[f"l{li}_wa"][:], a_sb[gi][:, :n]),
                     (WT[f"l{li}_corr"][:], d0_t[:1, :n])],
                    [WT[f"l{li}_nw{i}"] for i in range(1, 4)],
                    nB[:3])
                ln_apply(ps, n, nB[3], VT[f"l{li}_ng"],
                         x_sb[gi][:, :n], x_sb[gi][:, :n])

        # ---- decoder ---------------------------------------------------
        dWt = [WT[f"dec_w{i}"] for i in range(3)]
        dBt = [VT[f"dec_b{i}"] for i in range(3)]
        for gi, (c, n) in enumerate(ngrp):
            ps = mm_chain(n, [(dWt[0][:], x_sb[gi][:, :n])], [dWt[1]], [dBt[0]])
            h2 = hwork.tile([P, 512], f32r, tag="h")
            nc.scalar.activation(out=h2[:, :n], in_=ps[:, :n],
                                 func=AF.Relu, bias=dBt[1][:])
            ps3 = mmps.tile([P, 512], f32, tag="mm")
            nc.tensor.matmul(out=ps3[:OUT, :n], lhsT=dWt[2][:, :OUT],
                             rhs=h2[:, :n], start=True, stop=True)
            ob = hwork.tile([OUT, 512], f32, tag="h4b", name="ob")
            nc.scalar.activation(out=ob[:, :n], in_=ps3[:OUT, :n],
                                 func=AF.Identity, bias=dBt[2][:OUT, :])
            nc.sync.dma_start(out=d_out[:, c:c + n], in_=ob[:, :n])

    split_excess_waits(nc, maxw=1)
    return nc


# ---------------------------------------------------------------------------
# entry point
# ---------------------------------------------------------------------------

_CACHE = {}


def _ensure_ntff_hook():
    """The image's antenv lacks axon_hooks; recreate it and register the
    NTFF profile hook the same way trn_agent_boot.boot() would."""
    import types
    try:
        from antenv.axon_hooks import get_axon_ntff_profile_hook  # noqa: F401
        return
    except ImportError:
        pass
    try:
        import antenv
        from trn_agent_boot.trn_boot import _ntff_profile_via_ctypes
        mod = types.ModuleType("antenv.axon_hooks")
        _h = [None]
        mod.set_axon_ntff_profile_hook = lambda h: _h.__setitem__(0, h)
        mod.get_axon_ntff_profile_hook = lambda: _h[0]
        sys.modules["antenv.axon_hooks"] = mod
        antenv.axon_hooks = mod
        mod.set_axon_ntff_profile_hook(
            _ntff_profile_via_ctypes("/opt/axon/libaxon_pjrt.so"))
    except Exception as e:  # profiling is best-effort
        print(f"ntff hook setup failed: {e}", file=sys.stderr)


def _run(inputs, trace=False):
    pp = prep(**inputs)
    wf, vf, wb = pack_blobs(pp)
    key = (pp["NPAD"], pp["EPC"], pp["TB"], wf.data().shape[0], vf.data().shape[0])
    if key not in _CACHE:
        _CACHE[key] = build(pp, wf, vf)
    nc = _CACHE[key]

    wfd, vfd = wf.data(), vf.data()
    in_maps = []
    for c in range(NC):
        in_maps.append({
            "wf": wfd, "vf": vfd, "wb": wb,
            "eattr": pp["eattr"][c], "nattr": pp["nattrT"][c],
            "S": pp["S"][c], "oh_fm": pp["oh_fm"][c], "oh_em": pp["oh_em"][c],
            "deg0": pp["deg0"][c], "invc": pp["invc"],
        })
    if trace:
        _ensure_ntff_hook()
    res = run_bass_kernel_spmd(nc, in_maps, core_ids=list(range(NC)),
                               trace=trace)
    npc = pp["npc"]
    out = np.concatenate(
        [res.results[c]["out"][:, :npc].T for c in range(NC)], 0)
    return np.ascontiguousarray(out, dtype=np.float32), res


def kernel(**inputs):
    out, _ = _run(inputs, trace=False)
    return out


# revision 8
# speedup vs baseline: 1.0251x; 1.0251x over previous
"""MeshGraphNet forward on 8 Trainium2 NeuronCores (Bass/Tile).

Sharding: nodes in 8 contiguous blocks (batch is sorted, so graph segments
stay contiguous); edges colocated with their receiver node, grouped by
128-node block and padded so every block owns exactly T_B 128-edge tiles.
With edges partitioned by receiver, all message passing is core-local:
scatter-mean is a per-block one-hot matmul (1/deg folded into the one-hot);
the only cross-core exchange is a [128,4] AllReduce for global mean-pooling.

Layouts: activations are feature-major [128 feat, cols] for matmuls;
LayerNorm stats/apply detour through edge-major via PE transposes.
LN beta vectors are folded into downstream consumer biases on the host
(deg-0 receivers get a rank-1 correction matmul), so the device only
applies gamma. Matmuls run as float32r (full PE rate at N>=256); the
edge-latent residual state is stored bf16.
"""

import sys

for _p in ("/opt/trn_rl_repo",):
    if _p not in sys.path:
        sys.path.insert(0, _p)

from contextlib import ExitStack

import ml_dtypes
import numpy as np

import concourse.bass as bass
import concourse.mybir as mybir
import concourse.tile as tile
from concourse.bass_utils import run_bass_kernel_spmd
from concourse.masks import make_identity

P = 128
NC = 8
H = 128
NG = 4
OUT = 3
L = 4
LN_EPS = 1e-5

f32 = mybir.dt.float32
f32r = mybir.dt.float32r
bf16 = mybir.dt.bfloat16
AF = mybir.ActivationFunctionType
ALU = mybir.AluOpType
BF = ml_dtypes.bfloat16


def _np(x):
    return np.asarray(x, dtype=np.float32)


def split_excess_waits(nc, maxw=1):
    """Walrus here rejects >1 semaphore wait per instruction; hoist extra
    waits onto preceding NoOps on the same engine."""
    nsplit = 0
    for f in nc.m.functions:
        for blk in f.blocks:
            idx = 0
            insts = blk.instructions
            while idx < len(insts):
                ins = insts[idx]
                si = ins.sync_info
                if si is None or len(si.on_wait) <= maxw:
                    idx += 1
                    continue
                waits = list(si.on_wait)
                extra, keep = waits[:-maxw], waits[-maxw:]
                while extra:
                    chunk, extra = extra[:maxw], extra[maxw:]
                    nop = mybir.InstNoOp(
                        name=nc.get_next_instruction_name(), ins=[], outs=[])
                    nop.engine = ins.engine
                    nop.sync_info = mybir.SyncInfo(on_wait=chunk, on_update=[])
                    nc.register_instruction(nop)
                    insts.insert(idx, nop)
                    idx += 1
                    nsplit += 1
                si.on_wait = keep
                ins.sync_info = si
                idx += 1
    return nsplit


# ---------------------------------------------------------------------------
# host-side preprocessing
# ---------------------------------------------------------------------------

def prep(node_attr, edge_attr, edge_index, batch, params):
    node_attr = np.asarray(node_attr, np.float32)
    edge_attr = np.asarray(edge_attr, np.float32)
    edge_index = np.asarray(edge_index, np.int64)
    batch = np.asarray(batch, np.int64)

    N, NODE_IN = node_attr.shape
    E, EDGE_IN = edge_attr.shape
    assert N % NC == 0
    npc = N // NC
    NB = (npc + P - 1) // P
    NPAD = NB * P

    col = edge_index[1]
    deg = np.bincount(col, minlength=N).astype(np.float32)
    invdeg = (1.0 / np.maximum(deg, 1.0)).astype(np.float32)

    core = col // npc
    lid = col - core * npc
    blk = lid // P
    gb = core * NB + blk
    cnt = np.bincount(gb, minlength=NC * NB)
    TB = int(np.ceil(max(1, cnt.max()) / P))
    EPC = NB * TB * P

    order = np.argsort(gb, kind="stable")
    sgb = gb[order]
    block_starts = np.searchsorted(sgb, np.arange(NC * NB))
    pos = np.arange(E) - block_starts[sgb]
    slot = sgb * TB * P + pos

    eattr_pad = np.zeros((NC * EPC, EDGE_IN), np.float32)
    eattr_pad[slot] = edge_attr[order]
    eattr = np.ascontiguousarray(
        eattr_pad.reshape(NC, EPC, EDGE_IN).transpose(0, 2, 1))

    S = np.zeros((NC * EPC, P), np.float32)
    S[slot, lid[order] % P] = invdeg[col[order]]
    S = np.ascontiguousarray(S.reshape(NC, NB * TB, P, P).astype(BF))

    nattrT = np.zeros((NC, NODE_IN, NPAD), np.float32)
    oh_fm = np.zeros((NC, NG, NPAD), np.float32)
    oh_em = np.zeros((NC, NPAD, NG), np.float32)
    deg0 = np.zeros((NC, 1, NPAD), np.float32)
    for c in range(NC):
        nattrT[c, :, :npc] = node_attr[c * npc:(c + 1) * npc].T
        bc = batch[c * npc:(c + 1) * npc]
        oh = (bc[:, None] == np.arange(NG)[None, :]).astype(np.float32)
        oh_fm[c, :, :npc] = oh.T
        oh_em[c, :npc, :] = oh
        deg0[c, 0, :npc] = (deg[c * npc:(c + 1) * npc] == 0.0)

    gcnt = np.bincount(batch, minlength=NG).astype(np.float32)
    invc = (1.0 / np.maximum(gcnt, 1.0)).astype(np.float32)[:, None]

    # ---- weights with beta folding -------------------------------------
    def WB(p):
        return [_np(w) for w in p["W"]], [_np(b) for b in p["b"]]

    pg = params
    gW, gb_ = WB(pg["glob_mlp"])
    Wl, bl = _np(pg["glob_lin"][0]), _np(pg["glob_lin"][1])
    gW4 = gW[3] @ Wl
    gb4 = gb_[3] @ Wl + bl
    glob = (gW[:3] + [gW4], gb_[:3] + [gb4])

    neW, neB = WB(pg["node_enc"])
    ne_g, ne_beta = _np(pg["node_enc"]["g"]), _np(pg["node_enc"]["beta"])
    eeW, eeB = WB(pg["edge_enc"])
    ee_g, ee_beta = _np(pg["edge_enc"]["g"]), _np(pg["edge_enc"]["beta"])

    layers = []
    Ce = ee_beta.copy()          # constant folded out of e-state
    Cx = ne_beta.copy()          # constant folded out of x-state
    for lp in pg["layers"]:
        eW, eB = WB(lp["edge"])
        e_g, e_beta = _np(lp["edge"]["g"]), _np(lp["edge"]["beta"])
        nW, nB = WB(lp["node"])
        n_g, n_beta = _np(lp["node"]["g"]), _np(lp["node"]["beta"])
        eB0 = eB[0] + Ce @ eW[0]
        Ce = Ce + e_beta
        Wx, Wa = nW[0][:H], nW[0][H:]
        nB0 = nB[0] + Cx @ Wx + Ce @ Wa
        corr = -(Ce @ Wa)
        Cx = Cx + n_beta
        layers.append(dict(eW=eW, eB=[eB0] + eB[1:], eg=e_g,
                           Wx=Wx, Wa=Wa, nW=nW, nB=[nB0] + nB[1:], ng=n_g,
                           corr=corr))

    dW, dB = WB(pg["dec"])
    dB0 = dB[0] + Cx @ dW[0]
    dec = (dW, [dB0] + dB[1:])

    return dict(
        N=N, E=E, npc=npc, NB=NB, NPAD=NPAD, TB=TB, EPC=EPC,
        NODE_IN=NODE_IN, EDGE_IN=EDGE_IN,
        eattr=eattr, S=S, nattrT=nattrT, oh_fm=oh_fm, oh_em=oh_em,
        deg0=deg0, invc=invc,
        glob=glob, node_enc=(neW, neB, ne_g), edge_enc=(eeW, eeB, ee_g),
        layers=layers, dec=dec,
    )


# ---------------------------------------------------------------------------
# weight blob packing
# ---------------------------------------------------------------------------

class Blob:
    """Concatenate [K,<=128] float32 slabs into one [R,128] DRAM tensor."""

    def __init__(self):
        self.rows = []
        self.r = 0
        self.index = {}

    def add(self, name, arr):
        arr = np.asarray(arr, np.float32)
        if arr.ndim == 1:
            arr = arr[None, :]
        k, m = arr.shape
        pad = np.zeros((k, P), np.float32)
        pad[:, :m] = arr
        self.index[name] = (self.r, k, m)
        self.rows.append(pad)
        self.r += k

    def data(self):
        return (np.concatenate(self.rows, 0) if self.rows
                else np.zeros((1, P), np.float32))


def pack_blobs(pp):
    wf = Blob()   # float32r matmul weights (lhsT layout [K, dout])
    vf = Blob()   # float32 per-feature vectors (biases, gammas)

    gW, gB = pp["glob"]
    for i, w in enumerate(gW):
        wf.add(f"glob_w{i}", w)
        vf.add(f"glob_b{i}", gB[i])
    eeW, eeB, eeg = pp["edge_enc"]
    for i, w in enumerate(eeW):
        wf.add(f"eenc_w{i}", w)
        vf.add(f"eenc_b{i}", eeB[i])
    vf.add("eenc_g", eeg)
    neW, neB, neg = pp["node_enc"]
    wf.add("nenc_wna", neW[0][:pp["NODE_IN"]])
    wf.add("nenc_wgf", neW[0][pp["NODE_IN"]:])
    for i in range(1, 4):
        wf.add(f"nenc_w{i}", neW[i])
    for i in range(4):
        vf.add(f"nenc_b{i}", neB[i])
    vf.add("nenc_g", neg)
    wb_rows = []
    for li, lp in enumerate(pp["layers"]):
        wb_rows.append(np.asarray(lp["eW"][0], np.float32).astype(BF))
        for i in range(1, 4):
            wf.add(f"l{li}_ew{i}", lp["eW"][i])
        for i in range(4):
            vf.add(f"l{li}_eb{i}", lp["eB"][i])
        vf.add(f"l{li}_eg", lp["eg"])
        wf.add(f"l{li}_wx", lp["Wx"])
        wf.add(f"l{li}_wa", lp["Wa"])
        wf.add(f"l{li}_corr", lp["corr"])
        for i in range(1, 4):
            wf.add(f"l{li}_nw{i}", lp["nW"][i])
        for i in range(4):
            vf.add(f"l{li}_nb{i}", lp["nB"][i])
        vf.add(f"l{li}_ng", lp["ng"])
    dW, dB = pp["dec"]
    for i, w in enumerate(dW):
        wf.add(f"dec_w{i}", w)
        vf.add(f"dec_b{i}", dB[i])
    wb = np.concatenate(wb_rows, 0)  # [4*128, 128] bf16 (layer edge W1)
    return wf, vf, wb


# ---------------------------------------------------------------------------
# device kernel
# ---------------------------------------------------------------------------

def build(pp, wf, vf):
    NPAD, NB, TB, EPC = pp["NPAD"], pp["NB"], pp["TB"], pp["EPC"]
    NODE_IN, EDGE_IN = pp["NODE_IN"], pp["EDGE_IN"]

    def grps(total):
        out, c = [], 0
        while c < total:
            n = min(512, total - c)
            out.append((c, n))
            c += n
        return out

    egrp = grps(EPC)
    ngrp = grps(NPAD)

    nc = bass.Bass()
    d_wf = nc.declare_dram_parameter("wf", list(wf.data().shape), f32r, isOutput=False)
    d_vf = nc.declare_dram_parameter("vf", list(vf.data().shape), f32, isOutput=False)
    d_wb = nc.declare_dram_parameter("wb", [L * H, H], bf16, isOutput=False)
    d_ea = nc.declare_dram_parameter("eattr", [EDGE_IN, EPC], f32r, isOutput=False)
    d_na = nc.declare_dram_parameter("nattr", [NODE_IN, NPAD], f32r, isOutput=False)
    d_S = nc.declare_dram_parameter("S", [NB * TB, P, P], bf16, isOutput=False)
    d_ofm = nc.declare_dram_parameter("oh_fm", [NG, NPAD], f32r, isOutput=False)
    d_oem = nc.declare_dram_parameter("oh_em", [NPAD, NG], f32r, isOutput=False)
    d_d0 = nc.declare_dram_parameter("deg0", [1, NPAD], f32r, isOutput=False)
    d_ic = nc.declare_dram_parameter("invc", [NG, 1], f32, isOutput=False)
    d_out = nc.declare_dram_parameter("out", [OUT, NPAD], f32, isOutput=True)

    cc_in = nc.dram_tensor("cc_in", [P, NG], f32)
    cc_out = nc.dram_tensor("cc_out", [P, NG], f32)

    ctx = ExitStack()
    with ctx:
        tc = ctx.enter_context(tile.TileContext(nc))
        singles = ctx.enter_context(tc.tile_pool(name="singles", bufs=1))
        epool = ctx.enter_context(tc.tile_pool(name="e", bufs=1))
        xpool = ctx.enter_context(tc.tile_pool(name="x", bufs=1))
        apool = ctx.enter_context(tc.tile_pool(name="agg", bufs=1))
        inp = ctx.enter_context(tc.tile_pool(name="inp", bufs=3))
        hwork = ctx.enter_context(tc.tile_pool(name="hwork", bufs=3))
        stat = ctx.enter_context(tc.tile_pool(name="stat", bufs=4))
        spool = ctx.enter_context(tc.tile_pool(name="S", bufs=2))
        mmps = ctx.enter_context(tc.tile_pool(name="mmps", bufs=3, space="PSUM"))
        emps = ctx.enter_context(tc.tile_pool(name="emps", bufs=2, space="PSUM"))
        smps = ctx.enter_context(tc.tile_pool(name="smps", bufs=2, space="PSUM"))

        # ---- constants and weights ------------------------------------
        ident_f = singles.tile([P, P], f32)
        make_identity(nc, ident_f[:])
        ident_r = singles.tile([P, P], f32r)
        nc.vector.tensor_copy(out=ident_r[:], in_=ident_f[:])
        ident_b = singles.tile([P, P], bf16)
        nc.vector.tensor_copy(out=ident_b[:], in_=ident_f[:])
        eps_t = singles.tile([P, 1], f32)
        nc.vector.memset(eps_t[:], LN_EPS)

        WT = {}
        for name, (r, k, m) in wf.index.items():
            t = singles.tile([k, P], f32r, tag=f"wf_{name}", name=f"wf_{name}")
            nc.sync.dma_start(out=t[:], in_=d_wf[r:r + k, :])
            WT[name] = t
        VT = {}
        for name, (r, k, m) in vf.index.items():
            t = singles.tile([P, 1], f32, tag=f"vf_{name}", name=f"vf_{name}")
            nc.sync.dma_start(
                out=t[:m, :], in_=d_vf[r:r + 1, :m].rearrange("o k -> k o"))
            VT[name] = t
        WB1 = []
        for li in range(L):
            t = singles.tile([H, H], bf16, tag=f"wb_{li}", name=f"wb_{li}")
            nc.sync.dma_start(out=t[:], in_=d_wb[li * H:(li + 1) * H, :])
            WB1.append(t)

        oem_sb = singles.tile([P, NB, NG], f32r)
        nc.sync.dma_start(out=oem_sb[:],
                          in_=d_oem[:].rearrange("(b p) g -> p b g", p=P))
        ic_sb = singles.tile([NG, 1], f32)
        nc.sync.dma_start(out=ic_sb[:], in_=d_ic[:])

        e_sb = [epool.tile([P, n], bf16, tag=f"e{i}", name=f"e{i}")
                for i, (c, n) in enumerate(egrp)]
        x_sb = [xpool.tile([P, n], f32r, tag=f"x{i}", name=f"x{i}")
                for i, (c, n) in enumerate(ngrp)]
        a_sb = [apool.tile([P, n], f32r, tag=f"a{i}", name=f"a{i}")
                for i, (c, n) in enumerate(ngrp)]

        # ---- helpers ---------------------------------------------------
        relu_eng = ["act", "dve", "act"]

        def epilogue(dst, ps_ap, bias, relu, eng):
            if eng == "act" or not relu:
                nc.scalar.activation(out=dst, in_=ps_ap,
                                     func=AF.Relu if relu else AF.Identity,
                                     bias=bias[:])
            else:
                nc.vector.tensor_scalar(
                    out=dst, in0=ps_ap, scalar1=bias[:], scalar2=0.0,
                    op0=ALU.add, op1=ALU.max)

        def mm_chain(n, first_mms, Ws, Bs):
            """first_mms: [(lhsT_ap, rhs_ap)] accumulated; then ReLU+linear
            per W in Ws (bias Bs). Returns final psum tile [P,512]."""
            ps = mmps.tile([P, 512], f32, tag="mm")
            for j, (lh, rh) in enumerate(first_mms):
                nc.tensor.matmul(out=ps[:, :n], lhsT=lh, rhs=rh,
                                 start=(j == 0), stop=(j == len(first_mms) - 1))
            cur = ps
            for i, W in enumerate(Ws):
                h = hwork.tile([P, 512], f32r, tag="h")
                epilogue(h[:, :n], cur[:, :n], Bs[i], True, relu_eng[i % 3])
                ps2 = mmps.tile([P, 512], f32, tag="mm")
                nc.tensor.matmul(out=ps2[:, :n], lhsT=W[:], rhs=h[:, :n],
                                 start=True, stop=True)
                cur = ps2
            return cur

        def ln_apply(ps_h4, n, b4, g, resid, dst):
            """dst <- LayerNorm(ps_h4 + b4) * g (+ resid)."""
            nt = n // P
            h4b = hwork.tile([P, 512], bf16, tag="h4b")
            nc.scalar.activation(out=h4b[:, :n], in_=ps_h4[:, :n],
                                 func=AF.Identity, bias=b4[:])
            em = emps.tile([P, 512], bf16, tag="em")
            for i in range(nt):
                nc.tensor.transpose(out=em[:, i * P:(i + 1) * P],
                                    in_=h4b[:, i * P:(i + 1) * P],
                                    identity=ident_b[:])
            mv = stat.tile([P, 4, 2], f32, tag="mv")
            for i in range(nt):
                st = stat.tile([P, 6], f32, tag="st")
                nc.vector.bn_stats(out=st[:], in_=em[:, i * P:(i + 1) * P])
                nc.vector.bn_aggr(out=mv[:, i, :], in_=st[:])
            rstd = stat.tile([P, 4], f32, tag="rstd")
            murs = stat.tile([P, 4], f32, tag="murs")
            nc.scalar.activation(out=rstd[:, :nt], in_=mv[:, :nt, 1],
                                 func=AF.Sqrt, bias=eps_t[:])
            nc.vector.reciprocal(out=rstd[:, :nt], in_=rstd[:, :nt])
            nc.vector.tensor_tensor(out=murs[:, :nt], in0=mv[:, :nt, 0],
                                    in1=rstd[:, :nt], op=ALU.mult)
            tem = hwork.tile([P, 512], bf16, tag="tem")
            for i in range(nt):
                nc.vector.tensor_scalar(
                    out=tem[:, i * P:(i + 1) * P], in0=em[:, i * P:(i + 1) * P],
                    scalar1=rstd[:, i:i + 1], scalar2=murs[:, i:i + 1],
                    op0=ALU.mult, op1=ALU.subtract)
            tfm = mmps.tile([P, 512], bf16, tag="mm")
            for i in range(nt):
                nc.tensor.transpose(out=tfm[:, i * P:(i + 1) * P],
                                    in_=tem[:, i * P:(i + 1) * P],
                                    identity=ident_b[:])
            if resid is None:
                nc.vector.tensor_scalar(out=dst, in0=tfm[:, :n],
                                        scalar1=g[:], scalar2=None,
                                        op0=ALU.mult, op1=ALU.bypass)
            else:
                nc.vector.scalar_tensor_tensor(
                    out=dst, in0=tfm[:, :n], scalar=g[:], in1=resid,
                    op0=ALU.mult, op1=ALU.add)

        def load_inp(dram, c, n, kdim):
            t = inp.tile([kdim, 512], f32r, tag="inany", name="inany")
            nc.sync.dma_start(out=t[:, :n], in_=dram[:, c:c + n])
            return t

        # ---- global encoder -------------------------------------------
        partial = smps.tile([P, P], f32, tag="sm")
        for gi, (c, n) in enumerate(ngrp):
            na_t = load_inp(d_na, c, n, NODE_IN)
            ps = mm_chain(n, [(WT["glob_w0"][:], na_t[:, :n])],
                          [WT[f"glob_w{i}"] for i in range(1, 4)],
                          [VT[f"glob_b{i}"] for i in range(3)])
            xg = hwork.tile([P, 512], f32r, tag="h4b")
            nc.scalar.activation(out=xg[:, :n], in_=ps[:, :n],
                                 func=AF.Identity, bias=VT["glob_b3"][:])
            for i in range(n // P):
                b = (c // P) + i
                em = emps.tile([P, 512], f32r, tag="em")
                nc.tensor.transpose(out=em[:, :P], in_=xg[:, i * P:(i + 1) * P],
                                    identity=ident_r[:])
                emsb = hwork.tile([P, P], f32r, tag="emsb")
                nc.vector.tensor_copy(out=emsb[:], in_=em[:, :P])
                nc.tensor.matmul(out=partial[:, :NG], lhsT=emsb[:],
                                 rhs=oem_sb[:, b, :],
                                 start=(b == 0), stop=(b == NB - 1))
        partial_sb = stat.tile([P, NG], f32, tag="psb")
        nc.vector.tensor_copy(out=partial_sb[:], in_=partial[:, :NG])
        nc.sync.dma_start(out=cc_in[:], in_=partial_sb[:])
        nc.gpsimd.collective_compute(
            "AllReduce", ALU.add, ins=[cc_in[:]], outs=[cc_out[:]],
            replica_groups=[list(range(NC))])
        pooled = stat.tile([P, NG], f32r, tag="pool")
        nc.sync.dma_start(out=pooled[:], in_=cc_out[:].bitcast(f32r))
        pooledT_ps = smps.tile([P, P], f32r, tag="sm")
        nc.tensor.transpose(out=pooledT_ps[:NG, :], in_=pooled[:],
                            identity=ident_r[:])
        pooledT = stat.tile([NG, P], f32r, tag="poolT")
        nc.vector.tensor_scalar(out=pooledT[:], in0=pooledT_ps[:NG, :],
                                scalar1=ic_sb[:], scalar2=None,
                                op0=ALU.mult, op1=ALU.bypass)

        # ---- edge encoder ---------------------------------------------
        for gi, (c, n) in enumerate(egrp):
            ea_t = load_inp(d_ea, c, n, EDGE_IN)
            ps = mm_chain(n, [(WT["eenc_w0"][:], ea_t[:, :n])],
                          [WT[f"eenc_w{i}"] for i in range(1, 4)],
                          [VT[f"eenc_b{i}"] for i in range(3)])
            ln_apply(ps, n, VT["eenc_b3"], VT["eenc_g"], None, e_sb[gi][:, :n])

        # ---- node encoder ---------------------------------------------
        for gi, (c, n) in enumerate(ngrp):
            ofm_t = load_inp(d_ofm, c, n, NG)
            gf_ps = mmps.tile([P, 512], f32, tag="mm")
            nc.tensor.matmul(out=gf_ps[:, :n], lhsT=pooledT[:],
                             rhs=ofm_t[:, :n], start=True, stop=True)
            gf = hwork.tile([P, 512], f32r, tag="h4b", name="gf")
            nc.vector.tensor_copy(out=gf[:, :n], in_=gf_ps[:, :n])
            na_t = load_inp(d_na, c, n, NODE_IN)
            ps = mm_chain(
                n,
                [(WT["nenc_wna"][:], na_t[:, :n]),
                 (WT["nenc_wgf"][:], gf[:, :n])],
                [WT[f"nenc_w{i}"] for i in range(1, 4)],
                [VT[f"nenc_b{i}"] for i in range(3)])
            ln_apply(ps, n, VT["nenc_b3"], VT["nenc_g"], None, x_sb[gi][:, :n])

        # ---- message passing layers -----------------------------------
        for li in range(L):
            eB = [VT[f"l{li}_eb{i}"] for i in range(4)]
            eW = [WT[f"l{li}_ew{i}"] for i in range(1, 4)]
            for gi, (c, n) in enumerate(egrp):
                ps = mm_chain(n, [(WB1[li][:], e_sb[gi][:, :n])],
                              eW, eB[:3])
                ln_apply(ps, n, eB[3], VT[f"l{li}_eg"],
                         e_sb[gi][:, :n], e_sb[gi][:, :n])
            for b in range(NB):
                s_t = spool.tile([P, TB, P], bf16, tag="S")
                nc.sync.dma_start(
                    out=s_t[:],
                    in_=d_S[b * TB:(b + 1) * TB].rearrange("t p n -> p t n"))
                aps = smps.tile([P, P], f32, tag="sm")
                for j in range(TB):
                    t = b * TB + j
                    gi, off = (t * P) // 512, (t * P) % 512
                    em = emps.tile([P, 512], bf16, tag="em")
                    nc.tensor.transpose(out=em[:, :P],
                                        in_=e_sb[gi][:, off:off + P],
                                        identity=ident_b[:])
                    emsb = hwork.tile([P, P], bf16, tag="emsb")
                    nc.vector.tensor_copy(out=emsb[:], in_=em[:, :P])
                    nc.tensor.matmul(out=aps[:], lhsT=emsb[:],
                                     rhs=s_t[:, j, :],
                                     start=(j == 0), stop=(j == TB - 1))
                gi, off = (b * P) // 512, (b * P) % 512
                nc.vector.tensor_copy(out=a_sb[gi][:, off:off + P], in_=aps[:])
            nB = [VT[f"l{li}_nb{i}"] for i in range(4)]
            for gi, (c, n) in enumerate(ngrp):
                d0_t = load_inp(d_d0, c, n, 1)
                ps = mm_chain(
                    n,
                    [(WT[f"l{li}_wx"][:], x_sb[gi][:, :n]),
                     (WT[f"l{li}_wa"][:], a_sb[gi][:, :n]),
                     (WT[f"l{li}_corr"][:], d0_t[:1, :n])],
                    [WT[f"l{li}_nw{i}"] for i in range(1, 4)],
                    nB[:3])
                ln_apply(ps, n, nB[3], VT[f"l{li}_ng"],
                         x_sb[gi][:, :n], x_sb[gi][:, :n])

        # ---- decoder ---------------------------------------------------
        dWt = [WT[f"dec_w{i}"] for i in range(3)]
        dBt = [VT[f"dec_b{i}"] for i in range(3)]
        for gi, (c, n) in enumerate(ngrp):
            ps = mm_chain(n, [(dWt[0][:], x_sb[gi][:, :n])], [dWt[1]], [dBt[0]])
            h2 = hwork.tile([P, 512], f32r, tag="h")
            nc.scalar.activation(out=h2[:, :n], in_=ps[:, :n],
                                 func=AF.Relu, bias=dBt[1][:])
            ps3 = mmps.tile([P, 512], f32, tag="mm")
            nc.tensor.matmul(out=ps3[:OUT, :n], lhsT=dWt[2][:, :OUT],
                             rhs=h2[:, :n], start=True, stop=True)
            ob = hwork.tile([OUT, 512], f32, tag="h4b", name="ob")
            nc.scalar.activation(out=ob[:, :n], in_=ps3[:OUT, :n],
                                 func=AF.Identity, bias=dBt[2][:OUT, :])
            nc.sync.dma_start(out=d_out[:, c:c + n], in_=ob[:, :n])

    split_excess_waits(nc, maxw=1)
    return nc


# ---------------------------------------------------------------------------
# entry point
# ---------------------------------------------------------------------------

_CACHE = {}


def _ensure_ntff_hook():
    """The image's antenv lacks axon_hooks; recreate it and register the
    NTFF profile hook the same way trn_agent_boot.boot() would."""
    import types
    try:
        from antenv.axon_hooks import get_axon_ntff_profile_hook  # noqa: F401
        return
    except ImportError:
        pass
    try:
        import antenv
        from trn_agent_boot.trn_boot import _ntff_profile_via_ctypes
        mod = types.ModuleType("antenv.axon_hooks")
        _h = [None]
        mod.set_axon_ntff_profile_hook = lambda h: _h.__setitem__(0, h)
        mod.get_axon_ntff_profile_hook = lambda: _h[0]
        sys.modules["antenv.axon_hooks"] = mod
        antenv.axon_hooks = mod
        mod.set_axon_ntff_profile_hook(
            _ntff_profile_via_ctypes("/opt/axon/libaxon_pjrt.so"))
    except Exception as e:  # profiling is best-effort
        print(f"ntff hook setup failed: {e}", file=sys.stderr)


def _run(inputs, trace=False):
    pp = prep(**inputs)
    wf, vf, wb = pack_blobs(pp)
    key = (pp["NPAD"], pp["EPC"], pp["TB"], wf.data().shape[0], vf.data().shape[0])
    if key not in _CACHE:
        _CACHE[key] = build(pp, wf, vf)
    nc = _CACHE[key]

    wfd, vfd = wf.data(), vf.data()
    in_maps = []
    for c in range(NC):
        in_maps.append({
            "wf": wfd, "vf": vfd, "wb": wb,
            "eattr": pp["eattr"][c], "nattr": pp["nattrT"][c],
            "S": pp["S"][c], "oh_fm": pp["oh_fm"][c], "oh_em": pp["oh_em"][c],
            "deg0": pp["deg0"][c], "invc": pp["invc"],
        })
    if trace:
        _ensure_ntff_hook()
    res = run_bass_kernel_spmd(nc, in_maps, core_ids=list(range(NC)),
                               trace=trace)
    npc = pp["npc"]
    out = np.concatenate(
        [res.results[c]["out"][:, :npc].T for c in range(NC)], 0)
    return np.ascontiguousarray(out, dtype=np.float32), res


def kernel(**inputs):
    out, _ = _run(inputs, trace=False)
    return out


# revision 9
# speedup vs baseline: 1.0772x; 1.0508x over previous
"""MeshGraphNet forward on 8 Trainium2 NeuronCores (Bass/Tile).

Sharding: nodes in 8 contiguous blocks (batch is sorted, so graph segments
stay contiguous); edges colocated with their receiver node, grouped by
128-node block and padded so every block owns exactly T_B 128-edge tiles.
With edges partitioned by receiver, all message passing is core-local:
scatter-mean is a per-block one-hot matmul (1/deg folded into the one-hot);
the only cross-core exchange is a [128,4] AllReduce for global mean-pooling.

Layouts: activations are feature-major [128 feat, cols] for matmuls;
LayerNorm stats/apply detour through edge-major via PE transposes.
LN beta vectors are folded into downstream consumer biases on the host
(deg-0 receivers get a rank-1 correction matmul), so the device only
applies gamma. Matmuls run as float32r (full PE rate at N>=256); the
edge-latent residual state is stored bf16.
"""

import sys

for _p in ("/opt/trn_rl_repo",):
    if _p not in sys.path:
        sys.path.insert(0, _p)

from contextlib import ExitStack

import ml_dtypes
import numpy as np

import concourse.bass as bass
import concourse.mybir as mybir
import concourse.tile as tile
from concourse.bass_utils import run_bass_kernel_spmd
from concourse.masks import make_identity

P = 128
NC = 8
H = 128
NG = 4
OUT = 3
L = 4
LN_EPS = 1e-5

f32 = mybir.dt.float32
f32r = mybir.dt.float32r
bf16 = mybir.dt.bfloat16
AF = mybir.ActivationFunctionType
ALU = mybir.AluOpType
BF = ml_dtypes.bfloat16


def _np(x):
    return np.asarray(x, dtype=np.float32)


def split_excess_waits(nc, maxw=1):
    """Walrus here rejects >1 semaphore wait per instruction; hoist extra
    waits onto preceding NoOps on the same engine."""
    nsplit = 0
    for f in nc.m.functions:
        for blk in f.blocks:
            idx = 0
            insts = blk.instructions
            while idx < len(insts):
                ins = insts[idx]
                si = ins.sync_info
                if si is None or len(si.on_wait) <= maxw:
                    idx += 1
                    continue
                waits = list(si.on_wait)
                extra, keep = waits[:-maxw], waits[-maxw:]
                while extra:
                    chunk, extra = extra[:maxw], extra[maxw:]
                    nop = mybir.InstNoOp(
                        name=nc.get_next_instruction_name(), ins=[], outs=[])
                    nop.engine = ins.engine
                    nop.sync_info = mybir.SyncInfo(on_wait=chunk, on_update=[])
                    nc.register_instruction(nop)
                    insts.insert(idx, nop)
                    idx += 1
                    nsplit += 1
                si.on_wait = keep
                ins.sync_info = si
                idx += 1
    return nsplit


# ---------------------------------------------------------------------------
# host-side preprocessing
# ---------------------------------------------------------------------------

def prep(node_attr, edge_attr, edge_index, batch, params):
    node_attr = np.asarray(node_attr, np.float32)
    edge_attr = np.asarray(edge_attr, np.float32)
    edge_index = np.asarray(edge_index, np.int64)
    batch = np.asarray(batch, np.int64)

    N, NODE_IN = node_attr.shape
    E, EDGE_IN = edge_attr.shape
    assert N % NC == 0
    npc = N // NC
    NB = (npc + P - 1) // P
    NPAD = NB * P

    col = edge_index[1]
    deg = np.bincount(col, minlength=N).astype(np.float32)
    invdeg = (1.0 / np.maximum(deg, 1.0)).astype(np.float32)

    core = col // npc
    lid = col - core * npc
    blk = lid // P
    gb = core * NB + blk
    cnt = np.bincount(gb, minlength=NC * NB)
    TB = int(np.ceil(max(1, cnt.max()) / P))
    EPC = NB * TB * P

    order = np.argsort(gb, kind="stable")
    sgb = gb[order]
    block_starts = np.searchsorted(sgb, np.arange(NC * NB))
    pos = np.arange(E) - block_starts[sgb]
    slot = sgb * TB * P + pos

    eattr_pad = np.zeros((NC * EPC, EDGE_IN), np.float32)
    eattr_pad[slot] = edge_attr[order]
    eattr = np.ascontiguousarray(
        eattr_pad.reshape(NC, EPC, EDGE_IN).transpose(0, 2, 1))

    S = np.zeros((NC * EPC, P), np.float32)
    S[slot, lid[order] % P] = invdeg[col[order]]
    S = np.ascontiguousarray(S.reshape(NC, NB * TB, P, P).astype(BF))

    nattrT = np.zeros((NC, NODE_IN, NPAD), np.float32)
    oh_fm = np.zeros((NC, NG, NPAD), np.float32)
    oh_em = np.zeros((NC, NPAD, NG), np.float32)
    deg0 = np.zeros((NC, 1, NPAD), np.float32)
    for c in range(NC):
        nattrT[c, :, :npc] = node_attr[c * npc:(c + 1) * npc].T
        bc = batch[c * npc:(c + 1) * npc]
        oh = (bc[:, None] == np.arange(NG)[None, :]).astype(np.float32)
        oh_fm[c, :, :npc] = oh.T
        oh_em[c, :npc, :] = oh
        deg0[c, 0, :npc] = (deg[c * npc:(c + 1) * npc] == 0.0)

    gcnt = np.bincount(batch, minlength=NG).astype(np.float32)
    invc = (1.0 / np.maximum(gcnt, 1.0)).astype(np.float32)[:, None]

    # ---- weights with beta folding -------------------------------------
    def WB(p):
        return [_np(w) for w in p["W"]], [_np(b) for b in p["b"]]

    pg = params
    gW, gb_ = WB(pg["glob_mlp"])
    Wl, bl = _np(pg["glob_lin"][0]), _np(pg["glob_lin"][1])
    gW4 = gW[3] @ Wl
    gb4 = gb_[3] @ Wl + bl
    glob = (gW[:3] + [gW4], gb_[:3] + [gb4])

    neW, neB = WB(pg["node_enc"])
    ne_g, ne_beta = _np(pg["node_enc"]["g"]), _np(pg["node_enc"]["beta"])
    eeW, eeB = WB(pg["edge_enc"])
    ee_g, ee_beta = _np(pg["edge_enc"]["g"]), _np(pg["edge_enc"]["beta"])

    layers = []
    Ce = ee_beta.copy()          # constant folded out of e-state
    Cx = ne_beta.copy()          # constant folded out of x-state
    for lp in pg["layers"]:
        eW, eB = WB(lp["edge"])
        e_g, e_beta = _np(lp["edge"]["g"]), _np(lp["edge"]["beta"])
        nW, nB = WB(lp["node"])
        n_g, n_beta = _np(lp["node"]["g"]), _np(lp["node"]["beta"])
        eB0 = eB[0] + Ce @ eW[0]
        Ce = Ce + e_beta
        Wx, Wa = nW[0][:H], nW[0][H:]
        nB0 = nB[0] + Cx @ Wx + Ce @ Wa
        corr = -(Ce @ Wa)
        Cx = Cx + n_beta
        layers.append(dict(eW=eW, eB=[eB0] + eB[1:], eg=e_g,
                           Wx=Wx, Wa=Wa, nW=nW, nB=[nB0] + nB[1:], ng=n_g,
                           corr=corr))

    dW, dB = WB(pg["dec"])
    dB0 = dB[0] + Cx @ dW[0]
    dec = (dW, [dB0] + dB[1:])

    return dict(
        N=N, E=E, npc=npc, NB=NB, NPAD=NPAD, TB=TB, EPC=EPC,
        NODE_IN=NODE_IN, EDGE_IN=EDGE_IN,
        eattr=eattr, S=S, nattrT=nattrT, oh_fm=oh_fm, oh_em=oh_em,
        deg0=deg0, invc=invc,
        glob=glob, node_enc=(neW, neB, ne_g), edge_enc=(eeW, eeB, ee_g),
        layers=layers, dec=dec,
    )


# ---------------------------------------------------------------------------
# weight blob packing
# ---------------------------------------------------------------------------

class Blob:
    """Concatenate [K,<=128] float32 slabs into one [R,128] DRAM tensor."""

    def __init__(self):
        self.rows = []
        self.r = 0
        self.index = {}

    def add(self, name, arr):
        arr = np.asarray(arr, np.float32)
        if arr.ndim == 1:
            arr = arr[None, :]
        k, m = arr.shape
        pad = np.zeros((k, P), np.float32)
        pad[:, :m] = arr
        self.index[name] = (self.r, k, m)
        self.rows.append(pad)
        self.r += k

    def data(self):
        return (np.concatenate(self.rows, 0) if self.rows
                else np.zeros((1, P), np.float32))


def pack_blobs(pp):
    wf = Blob()   # float32r matmul weights (lhsT layout [K, dout])
    vf = Blob()   # float32 per-feature vectors (biases, gammas)
    wbf = Blob()  # bf16 matmul weights (edge-side MLPs)

    gW, gB = pp["glob"]
    for i, w in enumerate(gW):
        wf.add(f"glob_w{i}", w)
        vf.add(f"glob_b{i}", gB[i])
    eeW, eeB, eeg = pp["edge_enc"]
    for i, w in enumerate(eeW):
        wbf.add(f"eenc_w{i}", w)
        vf.add(f"eenc_b{i}", eeB[i])
    vf.add("eenc_g", eeg)
    neW, neB, neg = pp["node_enc"]
    wf.add("nenc_wna", neW[0][:pp["NODE_IN"]])
    wf.add("nenc_wgf", neW[0][pp["NODE_IN"]:])
    for i in range(1, 4):
        wf.add(f"nenc_w{i}", neW[i])
    for i in range(4):
        vf.add(f"nenc_b{i}", neB[i])
    vf.add("nenc_g", neg)
    wb_rows = []
    for li, lp in enumerate(pp["layers"]):
        wb_rows.append(np.asarray(lp["eW"][0], np.float32).astype(BF))
        for i in range(1, 4):
            wbf.add(f"l{li}_ew{i}", lp["eW"][i])
        for i in range(4):
            vf.add(f"l{li}_eb{i}", lp["eB"][i])
        vf.add(f"l{li}_eg", lp["eg"])
        wf.add(f"l{li}_wx", lp["Wx"])
        wf.add(f"l{li}_wa", lp["Wa"])
        wf.add(f"l{li}_corr", lp["corr"])
        for i in range(1, 4):
            wf.add(f"l{li}_nw{i}", lp["nW"][i])
        for i in range(4):
            vf.add(f"l{li}_nb{i}", lp["nB"][i])
        vf.add(f"l{li}_ng", lp["ng"])
    dW, dB = pp["dec"]
    for i, w in enumerate(dW):
        wf.add(f"dec_w{i}", w)
        vf.add(f"dec_b{i}", dB[i])
    wb = np.concatenate(wb_rows, 0)  # [4*128, 128] bf16 (layer edge W1)
    return wf, vf, wb, wbf


# ---------------------------------------------------------------------------
# device kernel
# ---------------------------------------------------------------------------

def build(pp, wf, vf, wbf):
    NPAD, NB, TB, EPC = pp["NPAD"], pp["NB"], pp["TB"], pp["EPC"]
    NODE_IN, EDGE_IN = pp["NODE_IN"], pp["EDGE_IN"]

    def grps(total):
        out, c = [], 0
        while c < total:
            n = min(512, total - c)
            out.append((c, n))
            c += n
        return out

    egrp = grps(EPC)
    ngrp = grps(NPAD)

    nc = bass.Bass()
    d_wf = nc.declare_dram_parameter("wf", list(wf.data().shape), f32r, isOutput=False)
    d_vf = nc.declare_dram_parameter("vf", list(vf.data().shape), f32, isOutput=False)
    d_wb = nc.declare_dram_parameter("wb", [L * H, H], bf16, isOutput=False)
    d_wbf = nc.declare_dram_parameter("wbf", list(wbf.data().shape), bf16, isOutput=False)
    d_ea = nc.declare_dram_parameter("eattr", [EDGE_IN, EPC], bf16, isOutput=False)
    d_na = nc.declare_dram_parameter("nattr", [NODE_IN, NPAD], f32r, isOutput=False)
    d_S = nc.declare_dram_parameter("S", [NB * TB, P, P], bf16, isOutput=False)
    d_ofm = nc.declare_dram_parameter("oh_fm", [NG, NPAD], f32r, isOutput=False)
    d_oem = nc.declare_dram_parameter("oh_em", [NPAD, NG], f32r, isOutput=False)
    d_d0 = nc.declare_dram_parameter("deg0", [1, NPAD], f32r, isOutput=False)
    d_ic = nc.declare_dram_parameter("invc", [NG, 1], f32, isOutput=False)
    d_out = nc.declare_dram_parameter("out", [OUT, NPAD], f32, isOutput=True)

    cc_in = nc.dram_tensor("cc_in", [P, NG], f32)
    cc_out = nc.dram_tensor("cc_out", [P, NG], f32)

    ctx = ExitStack()
    with ctx:
        tc = ctx.enter_context(tile.TileContext(nc))
        singles = ctx.enter_context(tc.tile_pool(name="singles", bufs=1))
        epool = ctx.enter_context(tc.tile_pool(name="e", bufs=1))
        xpool = ctx.enter_context(tc.tile_pool(name="x", bufs=1))
        apool = ctx.enter_context(tc.tile_pool(name="agg", bufs=1))
        inp = ctx.enter_context(tc.tile_pool(name="inp", bufs=3))
        hwork = ctx.enter_context(tc.tile_pool(name="hwork", bufs=3))
        stat = ctx.enter_context(tc.tile_pool(name="stat", bufs=4))
        spool = ctx.enter_context(tc.tile_pool(name="S", bufs=2))
        mmps = ctx.enter_context(tc.tile_pool(name="mmps", bufs=3, space="PSUM"))
        emps = ctx.enter_context(tc.tile_pool(name="emps", bufs=2, space="PSUM"))
        smps = ctx.enter_context(tc.tile_pool(name="smps", bufs=2, space="PSUM"))

        # ---- constants and weights ------------------------------------
        ident_f = singles.tile([P, P], f32)
        make_identity(nc, ident_f[:])
        ident_r = singles.tile([P, P], f32r)
        nc.vector.tensor_copy(out=ident_r[:], in_=ident_f[:])
        ident_b = singles.tile([P, P], bf16)
        nc.vector.tensor_copy(out=ident_b[:], in_=ident_f[:])
        eps_t = singles.tile([P, 1], f32)
        nc.vector.memset(eps_t[:], LN_EPS)

        WT = {}
        for name, (r, k, m) in wf.index.items():
            t = singles.tile([k, P], f32r, tag=f"wf_{name}", name=f"wf_{name}")
            nc.sync.dma_start(out=t[:], in_=d_wf[r:r + k, :])
            WT[name] = t
        VT = {}
        for name, (r, k, m) in vf.index.items():
            t = singles.tile([P, 1], f32, tag=f"vf_{name}", name=f"vf_{name}")
            nc.sync.dma_start(
                out=t[:m, :], in_=d_vf[r:r + 1, :m].rearrange("o k -> k o"))
            VT[name] = t
        WTB = {}
        for name, (r, k, m) in wbf.index.items():
            t = singles.tile([k, P], bf16, tag=f"wbf_{name}", name=f"wbf_{name}")
            nc.sync.dma_start(out=t[:], in_=d_wbf[r:r + k, :])
            WTB[name] = t
        WB1 = []
        for li in range(L):
            t = singles.tile([H, H], bf16, tag=f"wb_{li}", name=f"wb_{li}")
            nc.sync.dma_start(out=t[:], in_=d_wb[li * H:(li + 1) * H, :])
            WB1.append(t)

        oem_sb = singles.tile([P, NB, NG], f32r)
        nc.sync.dma_start(out=oem_sb[:],
                          in_=d_oem[:].rearrange("(b p) g -> p b g", p=P))
        ic_sb = singles.tile([NG, 1], f32)
        nc.sync.dma_start(out=ic_sb[:], in_=d_ic[:])

        e_sb = [epool.tile([P, n], bf16, tag=f"e{i}", name=f"e{i}")
                for i, (c, n) in enumerate(egrp)]
        x_sb = [xpool.tile([P, n], f32r, tag=f"x{i}", name=f"x{i}")
                for i, (c, n) in enumerate(ngrp)]
        a_sb = [apool.tile([P, n], f32r, tag=f"a{i}", name=f"a{i}")
                for i, (c, n) in enumerate(ngrp)]

        # ---- helpers ---------------------------------------------------
        relu_eng = ["act", "dve", "act"]

        def epilogue(dst, ps_ap, bias, relu, eng):
            if eng == "act" or not relu:
                nc.scalar.activation(out=dst, in_=ps_ap,
                                     func=AF.Relu if relu else AF.Identity,
                                     bias=bias[:])
            else:
                nc.vector.tensor_scalar(
                    out=dst, in0=ps_ap, scalar1=bias[:], scalar2=0.0,
                    op0=ALU.add, op1=ALU.max)

        def mm_chain(n, first_mms, Ws, Bs, hdt=f32r):
            """first_mms: [(lhsT_ap, rhs_ap)] accumulated; then ReLU+linear
            per W in Ws (bias Bs). Returns final psum tile [P,512]."""
            ps = mmps.tile([P, 512], f32, tag="mm")
            for j, (lh, rh) in enumerate(first_mms):
                nc.tensor.matmul(out=ps[:, :n], lhsT=lh, rhs=rh,
                                 start=(j == 0), stop=(j == len(first_mms) - 1))
            cur = ps
            for i, W in enumerate(Ws):
                h = hwork.tile([P, 512], hdt, tag="h", name="h")
                epilogue(h[:, :n], cur[:, :n], Bs[i], True, relu_eng[i % 3])
                ps2 = mmps.tile([P, 512], f32, tag="mm")
                nc.tensor.matmul(out=ps2[:, :n], lhsT=W[:], rhs=h[:, :n],
                                 start=True, stop=True)
                cur = ps2
            return cur

        def ln_apply(ps_h4, n, b4, g, resid, dst):
            """dst <- LayerNorm(ps_h4 + b4) * g (+ resid)."""
            nt = n // P
            h4b = hwork.tile([P, 512], bf16, tag="h4b")
            nc.scalar.activation(out=h4b[:, :n], in_=ps_h4[:, :n],
                                 func=AF.Identity, bias=b4[:])
            em = emps.tile([P, 512], bf16, tag="em")
            for i in range(nt):
                nc.tensor.transpose(out=em[:, i * P:(i + 1) * P],
                                    in_=h4b[:, i * P:(i + 1) * P],
                                    identity=ident_b[:])
            mv = stat.tile([P, 4, 2], f32, tag="mv")
            for i in range(nt):
                st = stat.tile([P, 6], f32, tag="st")
                nc.vector.bn_stats(out=st[:], in_=em[:, i * P:(i + 1) * P])
                nc.vector.bn_aggr(out=mv[:, i, :], in_=st[:])
            rstd = stat.tile([P, 4], f32, tag="rstd")
            murs = stat.tile([P, 4], f32, tag="murs")
            nc.scalar.activation(out=rstd[:, :nt], in_=mv[:, :nt, 1],
                                 func=AF.Sqrt, bias=eps_t[:])
            nc.vector.reciprocal(out=rstd[:, :nt], in_=rstd[:, :nt])
            nc.vector.tensor_tensor(out=murs[:, :nt], in0=mv[:, :nt, 0],
                                    in1=rstd[:, :nt], op=ALU.mult)
            tem = hwork.tile([P, 512], bf16, tag="tem")
            for i in range(nt):
                nc.vector.tensor_scalar(
                    out=tem[:, i * P:(i + 1) * P], in0=em[:, i * P:(i + 1) * P],
                    scalar1=rstd[:, i:i + 1], scalar2=murs[:, i:i + 1],
                    op0=ALU.mult, op1=ALU.subtract)
            tfm = mmps.tile([P, 512], bf16, tag="mm")
            for i in range(nt):
                nc.tensor.transpose(out=tfm[:, i * P:(i + 1) * P],
                                    in_=tem[:, i * P:(i + 1) * P],
                                    identity=ident_b[:])
            if resid is None:
                nc.vector.tensor_scalar(out=dst, in0=tfm[:, :n],
                                        scalar1=g[:], scalar2=None,
                                        op0=ALU.mult, op1=ALU.bypass)
            else:
                nc.vector.scalar_tensor_tensor(
                    out=dst, in0=tfm[:, :n], scalar=g[:], in1=resid,
                    op0=ALU.mult, op1=ALU.add)

        def load_inp(dram, c, n, kdim, dt=f32r):
            t = inp.tile([kdim, 512], dt, tag="inany", name="inany")
            nc.sync.dma_start(out=t[:, :n], in_=dram[:, c:c + n])
            return t

        # ---- global encoder -------------------------------------------
        partial = smps.tile([P, P], f32, tag="sm")
        for gi, (c, n) in enumerate(ngrp):
            na_t = load_inp(d_na, c, n, NODE_IN)
            ps = mm_chain(n, [(WT["glob_w0"][:], na_t[:, :n])],
                          [WT[f"glob_w{i}"] for i in range(1, 4)],
                          [VT[f"glob_b{i}"] for i in range(3)])
            xg = hwork.tile([P, 512], f32r, tag="h4b")
            nc.scalar.activation(out=xg[:, :n], in_=ps[:, :n],
                                 func=AF.Identity, bias=VT["glob_b3"][:])
            for i in range(n // P):
                b = (c // P) + i
                em = emps.tile([P, 512], f32r, tag="em")
                nc.tensor.transpose(out=em[:, :P], in_=xg[:, i * P:(i + 1) * P],
                                    identity=ident_r[:])
                emsb = hwork.tile([P, P], f32r, tag="emsb")
                nc.vector.tensor_copy(out=emsb[:], in_=em[:, :P])
                nc.tensor.matmul(out=partial[:, :NG], lhsT=emsb[:],
                                 rhs=oem_sb[:, b, :],
                                 start=(b == 0), stop=(b == NB - 1))
        partial_sb = stat.tile([P, NG], f32, tag="psb")
        nc.vector.tensor_copy(out=partial_sb[:], in_=partial[:, :NG])
        nc.sync.dma_start(out=cc_in[:], in_=partial_sb[:])
        nc.gpsimd.collective_compute(
            "AllReduce", ALU.add, ins=[cc_in[:]], outs=[cc_out[:]],
            replica_groups=[list(range(NC))])
        pooled = stat.tile([P, NG], f32r, tag="pool")
        nc.sync.dma_start(out=pooled[:], in_=cc_out[:].bitcast(f32r))
        pooledT_ps = smps.tile([P, P], f32r, tag="sm")
        nc.tensor.transpose(out=pooledT_ps[:NG, :], in_=pooled[:],
                            identity=ident_r[:])
        pooledT = stat.tile([NG, P], f32r, tag="poolT")
        nc.vector.tensor_scalar(out=pooledT[:], in0=pooledT_ps[:NG, :],
                                scalar1=ic_sb[:], scalar2=None,
                                op0=ALU.mult, op1=ALU.bypass)

        # ---- edge encoder ---------------------------------------------
        for gi, (c, n) in enumerate(egrp):
            ea_t = load_inp(d_ea, c, n, EDGE_IN, bf16)
            ps = mm_chain(n, [(WTB["eenc_w0"][:], ea_t[:, :n])],
                          [WTB[f"eenc_w{i}"] for i in range(1, 4)],
                          [VT[f"eenc_b{i}"] for i in range(3)], hdt=bf16)
            ln_apply(ps, n, VT["eenc_b3"], VT["eenc_g"], None, e_sb[gi][:, :n])

        # ---- node encoder ---------------------------------------------
        for gi, (c, n) in enumerate(ngrp):
            ofm_t = load_inp(d_ofm, c, n, NG)
            gf_ps = mmps.tile([P, 512], f32, tag="mm")
            nc.tensor.matmul(out=gf_ps[:, :n], lhsT=pooledT[:],
                             rhs=ofm_t[:, :n], start=True, stop=True)
            gf = hwork.tile([P, 512], f32r, tag="h4b", name="gf")
            nc.vector.tensor_copy(out=gf[:, :n], in_=gf_ps[:, :n])
            na_t = load_inp(d_na, c, n, NODE_IN)
            ps = mm_chain(
                n,
                [(WT["nenc_wna"][:], na_t[:, :n]),
                 (WT["nenc_wgf"][:], gf[:, :n])],
                [WT[f"nenc_w{i}"] for i in range(1, 4)],
                [VT[f"nenc_b{i}"] for i in range(3)])
            ln_apply(ps, n, VT["nenc_b3"], VT["nenc_g"], None, x_sb[gi][:, :n])

        # ---- message passing layers -----------------------------------
        for li in range(L):
            eB = [VT[f"l{li}_eb{i}"] for i in range(4)]
            eW = [WTB[f"l{li}_ew{i}"] for i in range(1, 4)]
            for gi, (c, n) in enumerate(egrp):
                ps = mm_chain(n, [(WB1[li][:], e_sb[gi][:, :n])],
                              eW, eB[:3], hdt=bf16)
                ln_apply(ps, n, eB[3], VT[f"l{li}_eg"],
                         e_sb[gi][:, :n], e_sb[gi][:, :n])
            for b in range(NB):
                s_t = spool.tile([P, TB, P], bf16, tag="S")
                nc.sync.dma_start(
                    out=s_t[:],
                    in_=d_S[b * TB:(b + 1) * TB].rearrange("t p n -> p t n"))
                aps = smps.tile([P, P], f32, tag="sm")
                for j in range(TB):
                    t = b * TB + j
                    gi, off = (t * P) // 512, (t * P) % 512
                    em = emps.tile([P, 512], bf16, tag="em")
                    nc.tensor.transpose(out=em[:, :P],
                                        in_=e_sb[gi][:, off:off + P],
                                        identity=ident_b[:])
                    emsb = hwork.tile([P, P], bf16, tag="emsb")
                    nc.vector.tensor_copy(out=emsb[:], in_=em[:, :P])
                    nc.tensor.matmul(out=aps[:], lhsT=emsb[:],
                                     rhs=s_t[:, j, :],
                                     start=(j == 0), stop=(j == TB - 1))
                gi, off = (b * P) // 512, (b * P) % 512
                nc.vector.tensor_copy(out=a_sb[gi][:, off:off + P], in_=aps[:])
            nB = [VT[f"l{li}_nb{i}"] for i in range(4)]
            for gi, (c, n) in enumerate(ngrp):
                d0_t = load_inp(d_d0, c, n, 1)
                ps = mm_chain(
                    n,
                    [(WT[f"l{li}_wx"][:], x_sb[gi][:, :n]),
                     (WT[f"l{li}_wa"][:], a_sb[gi][:, :n]),
                     (WT[f"l{li}_corr"][:], d0_t[:1, :n])],
                    [WT[f"l{li}_nw{i}"] for i in range(1, 4)],
                    nB[:3])
                ln_apply(ps, n, nB[3], VT[f"l{li}_ng"],
                         x_sb[gi][:, :n], x_sb[gi][:, :n])

        # ---- decoder ---------------------------------------------------
        dWt = [WT[f"dec_w{i}"] for i in range(3)]
        dBt = [VT[f"dec_b{i}"] for i in range(3)]
        for gi, (c, n) in enumerate(ngrp):
            ps = mm_chain(n, [(dWt[0][:], x_sb[gi][:, :n])], [dWt[1]], [dBt[0]])
            h2 = hwork.tile([P, 512], f32r, tag="h")
            nc.scalar.activation(out=h2[:, :n], in_=ps[:, :n],
                                 func=AF.Relu, bias=dBt[1][:])
            ps3 = mmps.tile([P, 512], f32, tag="mm")
            nc.tensor.matmul(out=ps3[:OUT, :n], lhsT=dWt[2][:, :OUT],
                             rhs=h2[:, :n], start=True, stop=True)
            ob = hwork.tile([OUT, 512], f32, tag="h4b", name="ob")
            nc.scalar.activation(out=ob[:, :n], in_=ps3[:OUT, :n],
                                 func=AF.Identity, bias=dBt[2][:OUT, :])
            nc.sync.dma_start(out=d_out[:, c:c + n], in_=ob[:, :n])

    split_excess_waits(nc, maxw=1)
    return nc


# ---------------------------------------------------------------------------
# entry point
# ---------------------------------------------------------------------------

_CACHE = {}


def _ensure_ntff_hook():
    """The image's antenv lacks axon_hooks; recreate it and register the
    NTFF profile hook the same way trn_agent_boot.boot() would."""
    import types
    try:
        from antenv.axon_hooks import get_axon_ntff_profile_hook  # noqa: F401
        return
    except ImportError:
        pass
    try:
        import antenv
        from trn_agent_boot.trn_boot import _ntff_profile_via_ctypes
        mod = types.ModuleType("antenv.axon_hooks")
        _h = [None]
        mod.set_axon_ntff_profile_hook = lambda h: _h.__setitem__(0, h)
        mod.get_axon_ntff_profile_hook = lambda: _h[0]
        sys.modules["antenv.axon_hooks"] = mod
        antenv.axon_hooks = mod
        mod.set_axon_ntff_profile_hook(
            _ntff_profile_via_ctypes("/opt/axon/libaxon_pjrt.so"))
    except Exception as e:  # profiling is best-effort
        print(f"ntff hook setup failed: {e}", file=sys.stderr)


def _run(inputs, trace=False):
    pp = prep(**inputs)
    wf, vf, wb, wbf = pack_blobs(pp)
    key = (pp["NPAD"], pp["EPC"], pp["TB"], wf.data().shape[0], vf.data().shape[0])
    if key not in _CACHE:
        _CACHE[key] = build(pp, wf, vf, wbf)
    nc = _CACHE[key]

    wfd, vfd = wf.data(), vf.data()
    wbfd = wbf.data().astype(BF)
    in_maps = []
    for c in range(NC):
        in_maps.append({
            "wf": wfd, "vf": vfd, "wb": wb, "wbf": wbfd,
            "eattr": pp["eattr"][c].astype(BF), "nattr": pp["nattrT"][c],
            "S": pp["S"][c], "oh_fm": pp["oh_fm"][c], "oh_em": pp["oh_em"][c],
            "deg0": pp["deg0"][c], "invc": pp["invc"],
        })
    if trace:
        _ensure_ntff_hook()
    res = run_bass_kernel_spmd(nc, in_maps, core_ids=list(range(NC)),
                               trace=trace)
    npc = pp["npc"]
    out = np.concatenate(
        [res.results[c]["out"][:, :npc].T for c in range(NC)], 0)
    return np.ascontiguousarray(out, dtype=np.float32), res


def kernel(**inputs):
    out, _ = _run(inputs, trace=False)
    return out


# revision 10
# speedup vs baseline: 1.0844x; 1.0066x over previous
"""MeshGraphNet forward on 8 Trainium2 NeuronCores (Bass/Tile).

Sharding: nodes in 8 contiguous blocks (batch is sorted, so graph segments
stay contiguous); edges colocated with their receiver node, grouped by
128-node block and padded so every block owns exactly T_B 128-edge tiles.
With edges partitioned by receiver, all message passing is core-local:
scatter-mean is a per-block one-hot matmul (1/deg folded into the one-hot);
the only cross-core exchange is a [128,4] AllReduce for global mean-pooling.

Layouts: activations are feature-major [128 feat, cols] for matmuls;
LayerNorm stats/apply detour through edge-major via PE transposes.
LN beta vectors are folded into downstream consumer biases on the host
(deg-0 receivers get a rank-1 correction matmul), so the device only
applies gamma. Matmuls run as float32r (full PE rate at N>=256); the
edge-latent residual state is stored bf16.
"""

import sys

for _p in ("/opt/trn_rl_repo",):
    if _p not in sys.path:
        sys.path.insert(0, _p)

from contextlib import ExitStack

import ml_dtypes
import numpy as np

import concourse.bass as bass
import concourse.mybir as mybir
import concourse.tile as tile
from concourse.bass_utils import run_bass_kernel_spmd
from concourse.masks import make_identity

P = 128
NC = 8
H = 128
NG = 4
OUT = 3
L = 4
LN_EPS = 1e-5

f32 = mybir.dt.float32
f32r = mybir.dt.float32r
bf16 = mybir.dt.bfloat16
AF = mybir.ActivationFunctionType
ALU = mybir.AluOpType
BF = ml_dtypes.bfloat16


def _np(x):
    return np.asarray(x, dtype=np.float32)


def split_excess_waits(nc, maxw=1):
    """Walrus here rejects >1 semaphore wait per instruction; hoist extra
    waits onto preceding NoOps on the same engine."""
    nsplit = 0
    for f in nc.m.functions:
        for blk in f.blocks:
            idx = 0
            insts = blk.instructions
            while idx < len(insts):
                ins = insts[idx]
                si = ins.sync_info
                if si is None or len(si.on_wait) <= maxw:
                    idx += 1
                    continue
                waits = list(si.on_wait)
                extra, keep = waits[:-maxw], waits[-maxw:]
                while extra:
                    chunk, extra = extra[:maxw], extra[maxw:]
                    nop = mybir.InstNoOp(
                        name=nc.get_next_instruction_name(), ins=[], outs=[])
                    nop.engine = ins.engine
                    nop.sync_info = mybir.SyncInfo(on_wait=chunk, on_update=[])
                    nc.register_instruction(nop)
                    insts.insert(idx, nop)
                    idx += 1
                    nsplit += 1
                si.on_wait = keep
                ins.sync_info = si
                idx += 1
    return nsplit


# ---------------------------------------------------------------------------
# host-side preprocessing
# ---------------------------------------------------------------------------

def prep(node_attr, edge_attr, edge_index, batch, params):
    node_attr = np.asarray(node_attr, np.float32)
    edge_attr = np.asarray(edge_attr, np.float32)
    edge_index = np.asarray(edge_index, np.int64)
    batch = np.asarray(batch, np.int64)

    N, NODE_IN = node_attr.shape
    E, EDGE_IN = edge_attr.shape
    assert N % NC == 0
    npc = N // NC
    NB = (npc + P - 1) // P
    NPAD = NB * P

    col = edge_index[1]
    deg = np.bincount(col, minlength=N).astype(np.float32)
    invdeg = (1.0 / np.maximum(deg, 1.0)).astype(np.float32)

    core = col // npc
    lid = col - core * npc
    blk = lid // P
    gb = core * NB + blk
    cnt = np.bincount(gb, minlength=NC * NB)
    TB = int(np.ceil(max(1, cnt.max()) / P))
    EPC = NB * TB * P

    order = np.argsort(gb, kind="stable")
    sgb = gb[order]
    block_starts = np.searchsorted(sgb, np.arange(NC * NB))
    pos = np.arange(E) - block_starts[sgb]
    slot = sgb * TB * P + pos

    eattr_pad = np.zeros((NC * EPC, EDGE_IN), np.float32)
    eattr_pad[slot] = edge_attr[order]
    eattr = np.ascontiguousarray(
        eattr_pad.reshape(NC, EPC, EDGE_IN).transpose(0, 2, 1))

    S = np.zeros((NC * EPC, P), np.float32)
    S[slot, lid[order] % P] = invdeg[col[order]]
    S = np.ascontiguousarray(S.reshape(NC, NB * TB, P, P).astype(BF))

    nattrT = np.zeros((NC, NODE_IN, NPAD), np.float32)
    oh_fm = np.zeros((NC, NG, NPAD), np.float32)
    oh_em = np.zeros((NC, NPAD, NG), np.float32)
    deg0 = np.zeros((NC, 1, NPAD), np.float32)
    for c in range(NC):
        nattrT[c, :, :npc] = node_attr[c * npc:(c + 1) * npc].T
        bc = batch[c * npc:(c + 1) * npc]
        oh = (bc[:, None] == np.arange(NG)[None, :]).astype(np.float32)
        oh_fm[c, :, :npc] = oh.T
        oh_em[c, :npc, :] = oh
        deg0[c, 0, :npc] = (deg[c * npc:(c + 1) * npc] == 0.0)

    gcnt = np.bincount(batch, minlength=NG).astype(np.float32)
    invc = (1.0 / np.maximum(gcnt, 1.0)).astype(np.float32)[:, None]

    # ---- weights with beta folding -------------------------------------
    def WB(p):
        return [_np(w) for w in p["W"]], [_np(b) for b in p["b"]]

    pg = params
    gW, gb_ = WB(pg["glob_mlp"])
    Wl, bl = _np(pg["glob_lin"][0]), _np(pg["glob_lin"][1])
    gW4 = gW[3] @ Wl
    gb4 = gb_[3] @ Wl + bl
    glob = (gW[:3] + [gW4], gb_[:3] + [gb4])

    neW, neB = WB(pg["node_enc"])
    ne_g, ne_beta = _np(pg["node_enc"]["g"]), _np(pg["node_enc"]["beta"])
    eeW, eeB = WB(pg["edge_enc"])
    ee_g, ee_beta = _np(pg["edge_enc"]["g"]), _np(pg["edge_enc"]["beta"])

    layers = []
    Ce = ee_beta.copy()          # constant folded out of e-state
    Cx = ne_beta.copy()          # constant folded out of x-state
    for lp in pg["layers"]:
        eW, eB = WB(lp["edge"])
        e_g, e_beta = _np(lp["edge"]["g"]), _np(lp["edge"]["beta"])
        nW, nB = WB(lp["node"])
        n_g, n_beta = _np(lp["node"]["g"]), _np(lp["node"]["beta"])
        eB0 = eB[0] + Ce @ eW[0]
        Ce = Ce + e_beta
        Wx, Wa = nW[0][:H], nW[0][H:]
        nB0 = nB[0] + Cx @ Wx + Ce @ Wa
        corr = -(Ce @ Wa)
        Cx = Cx + n_beta
        layers.append(dict(eW=eW, eB=[eB0] + eB[1:], eg=e_g,
                           Wx=Wx, Wa=Wa, nW=nW, nB=[nB0] + nB[1:], ng=n_g,
                           corr=corr))

    dW, dB = WB(pg["dec"])
    dB0 = dB[0] + Cx @ dW[0]
    dec = (dW, [dB0] + dB[1:])

    return dict(
        N=N, E=E, npc=npc, NB=NB, NPAD=NPAD, TB=TB, EPC=EPC,
        NODE_IN=NODE_IN, EDGE_IN=EDGE_IN,
        eattr=eattr, S=S, nattrT=nattrT, oh_fm=oh_fm, oh_em=oh_em,
        deg0=deg0, invc=invc,
        glob=glob, node_enc=(neW, neB, ne_g), edge_enc=(eeW, eeB, ee_g),
        layers=layers, dec=dec,
    )


# ---------------------------------------------------------------------------
# weight blob packing
# ---------------------------------------------------------------------------

class Blob:
    """Concatenate [K,<=128] float32 slabs into one [R,128] DRAM tensor."""

    def __init__(self):
        self.rows = []
        self.r = 0
        self.index = {}

    def add(self, name, arr):
        arr = np.asarray(arr, np.float32)
        if arr.ndim == 1:
            arr = arr[None, :]
        k, m = arr.shape
        pad = np.zeros((k, P), np.float32)
        pad[:, :m] = arr
        self.index[name] = (self.r, k, m)
        self.rows.append(pad)
        self.r += k

    def data(self):
        return (np.concatenate(self.rows, 0) if self.rows
                else np.zeros((1, P), np.float32))


def pack_blobs(pp):
    wf = Blob()   # float32r matmul weights (lhsT layout [K, dout])
    vf = Blob()   # float32 per-feature vectors (biases, gammas)
    wbf = Blob()  # bf16 matmul weights (edge-side MLPs)

    gW, gB = pp["glob"]
    for i, w in enumerate(gW):
        wf.add(f"glob_w{i}", w)
        vf.add(f"glob_b{i}", gB[i])
    eeW, eeB, eeg = pp["edge_enc"]
    for i, w in enumerate(eeW):
        wbf.add(f"eenc_w{i}", w)
        vf.add(f"eenc_b{i}", eeB[i])
    vf.add("eenc_g", eeg)
    neW, neB, neg = pp["node_enc"]
    wf.add("nenc_wna", neW[0][:pp["NODE_IN"]])
    wf.add("nenc_wgf", neW[0][pp["NODE_IN"]:])
    for i in range(1, 4):
        wf.add(f"nenc_w{i}", neW[i])
    for i in range(4):
        vf.add(f"nenc_b{i}", neB[i])
    vf.add("nenc_g", neg)
    wb_rows = []
    for li, lp in enumerate(pp["layers"]):
        wb_rows.append(np.asarray(lp["eW"][0], np.float32).astype(BF))
        for i in range(1, 4):
            wbf.add(f"l{li}_ew{i}", lp["eW"][i])
        for i in range(4):
            vf.add(f"l{li}_eb{i}", lp["eB"][i])
        vf.add(f"l{li}_eg", lp["eg"])
        wf.add(f"l{li}_wx", lp["Wx"])
        wf.add(f"l{li}_wa", lp["Wa"])
        wf.add(f"l{li}_corr", lp["corr"])
        for i in range(1, 4):
            wf.add(f"l{li}_nw{i}", lp["nW"][i])
        for i in range(4):
            vf.add(f"l{li}_nb{i}", lp["nB"][i])
        vf.add(f"l{li}_ng", lp["ng"])
    dW, dB = pp["dec"]
    for i, w in enumerate(dW):
        wf.add(f"dec_w{i}", w)
        vf.add(f"dec_b{i}", dB[i])
    wb = np.concatenate(wb_rows, 0)  # [4*128, 128] bf16 (layer edge W1)
    return wf, vf, wb, wbf


# ---------------------------------------------------------------------------
# device kernel
# ---------------------------------------------------------------------------

def build(pp, wf, vf, wbf):
    NPAD, NB, TB, EPC = pp["NPAD"], pp["NB"], pp["TB"], pp["EPC"]
    NODE_IN, EDGE_IN = pp["NODE_IN"], pp["EDGE_IN"]

    def grps(total):
        out, c = [], 0
        while c < total:
            n = min(512, total - c)
            out.append((c, n))
            c += n
        return out

    egrp = grps(EPC)
    ngrp = grps(NPAD)

    nc = bass.Bass()
    d_wf = nc.declare_dram_parameter("wf", list(wf.data().shape), f32r, isOutput=False)
    d_vf = nc.declare_dram_parameter("vf", list(vf.data().shape), f32, isOutput=False)
    d_wb = nc.declare_dram_parameter("wb", [L * H, H], bf16, isOutput=False)
    d_wbf = nc.declare_dram_parameter("wbf", list(wbf.data().shape), bf16, isOutput=False)
    d_ea = nc.declare_dram_parameter("eattr", [EDGE_IN, EPC], bf16, isOutput=False)
    d_na = nc.declare_dram_parameter("nattr", [NODE_IN, NPAD], f32r, isOutput=False)
    d_S = nc.declare_dram_parameter("S", [NB * TB, P, P], bf16, isOutput=False)
    d_ofm = nc.declare_dram_parameter("oh_fm", [NG, NPAD], f32r, isOutput=False)
    d_oem = nc.declare_dram_parameter("oh_em", [NPAD, NG], f32r, isOutput=False)
    d_d0 = nc.declare_dram_parameter("deg0", [1, NPAD], f32r, isOutput=False)
    d_ic = nc.declare_dram_parameter("invc", [NG, 1], f32, isOutput=False)
    d_out = nc.declare_dram_parameter("out", [OUT, NPAD], f32, isOutput=True)

    cc_in = nc.dram_tensor("cc_in", [P, NG], f32)
    cc_out = nc.dram_tensor("cc_out", [P, NG], f32)

    ctx = ExitStack()
    with ctx:
        tc = ctx.enter_context(tile.TileContext(nc))
        singles = ctx.enter_context(tc.tile_pool(name="singles", bufs=1))
        epool = ctx.enter_context(tc.tile_pool(name="e", bufs=1))
        xpool = ctx.enter_context(tc.tile_pool(name="x", bufs=1))
        apool = ctx.enter_context(tc.tile_pool(name="agg", bufs=1))
        inp = ctx.enter_context(tc.tile_pool(name="inp", bufs=3))
        hwork = ctx.enter_context(tc.tile_pool(name="hwork", bufs=5))
        stat = ctx.enter_context(tc.tile_pool(name="stat", bufs=4))
        spool = ctx.enter_context(tc.tile_pool(name="S", bufs=2))
        mmps = ctx.enter_context(tc.tile_pool(name="mmps", bufs=4, space="PSUM"))
        emps = ctx.enter_context(tc.tile_pool(name="emps", bufs=2, space="PSUM"))
        smps = ctx.enter_context(tc.tile_pool(name="smps", bufs=2, space="PSUM"))

        # ---- constants and weights ------------------------------------
        ident_f = singles.tile([P, P], f32)
        make_identity(nc, ident_f[:])
        ident_r = singles.tile([P, P], f32r)
        nc.vector.tensor_copy(out=ident_r[:], in_=ident_f[:])
        ident_b = singles.tile([P, P], bf16)
        nc.vector.tensor_copy(out=ident_b[:], in_=ident_f[:])
        eps_t = singles.tile([P, 1], f32)
        nc.vector.memset(eps_t[:], LN_EPS)

        WT = {}
        for name, (r, k, m) in wf.index.items():
            t = singles.tile([k, P], f32r, tag=f"wf_{name}", name=f"wf_{name}")
            nc.sync.dma_start(out=t[:], in_=d_wf[r:r + k, :])
            WT[name] = t
        VT = {}
        for name, (r, k, m) in vf.index.items():
            t = singles.tile([P, 1], f32, tag=f"vf_{name}", name=f"vf_{name}")
            nc.sync.dma_start(
                out=t[:m, :], in_=d_vf[r:r + 1, :m].rearrange("o k -> k o"))
            VT[name] = t
        WTB = {}
        for name, (r, k, m) in wbf.index.items():
            t = singles.tile([k, P], bf16, tag=f"wbf_{name}", name=f"wbf_{name}")
            nc.sync.dma_start(out=t[:], in_=d_wbf[r:r + k, :])
            WTB[name] = t
        WB1 = []
        for li in range(L):
            t = singles.tile([H, H], bf16, tag=f"wb_{li}", name=f"wb_{li}")
            nc.sync.dma_start(out=t[:], in_=d_wb[li * H:(li + 1) * H, :])
            WB1.append(t)

        oem_sb = singles.tile([P, NB, NG], f32r)
        nc.sync.dma_start(out=oem_sb[:],
                          in_=d_oem[:].rearrange("(b p) g -> p b g", p=P))
        ic_sb = singles.tile([NG, 1], f32)
        nc.sync.dma_start(out=ic_sb[:], in_=d_ic[:])

        e_sb = [epool.tile([P, n], bf16, tag=f"e{i}", name=f"e{i}")
                for i, (c, n) in enumerate(egrp)]
        x_sb = [xpool.tile([P, n], f32r, tag=f"x{i}", name=f"x{i}")
                for i, (c, n) in enumerate(ngrp)]
        a_sb = [apool.tile([P, n], f32r, tag=f"a{i}", name=f"a{i}")
                for i, (c, n) in enumerate(ngrp)]

        # ---- helpers ---------------------------------------------------
        relu_eng = ["act", "dve", "act"]

        def epilogue(dst, ps_ap, bias, relu, eng):
            if eng == "act" or not relu:
                nc.scalar.activation(out=dst, in_=ps_ap,
                                     func=AF.Relu if relu else AF.Identity,
                                     bias=bias[:])
            else:
                nc.vector.tensor_scalar(
                    out=dst, in0=ps_ap, scalar1=bias[:], scalar2=0.0,
                    op0=ALU.add, op1=ALU.max)

        def mm_chain(n, first_mms, Ws, Bs, hdt=f32r):
            """first_mms: [(lhsT_ap, rhs_ap)] accumulated; then ReLU+linear
            per W in Ws (bias Bs). Returns final psum tile [P,512]."""
            ps = mmps.tile([P, 512], f32, tag="mm")
            for j, (lh, rh) in enumerate(first_mms):
                nc.tensor.matmul(out=ps[:, :n], lhsT=lh, rhs=rh,
                                 start=(j == 0), stop=(j == len(first_mms) - 1))
            cur = ps
            for i, W in enumerate(Ws):
                h = hwork.tile([P, 512], hdt, tag="h", name="h")
                epilogue(h[:, :n], cur[:, :n], Bs[i], True, relu_eng[i % 3])
                ps2 = mmps.tile([P, 512], f32, tag="mm")
                nc.tensor.matmul(out=ps2[:, :n], lhsT=W[:], rhs=h[:, :n],
                                 start=True, stop=True)
                cur = ps2
            return cur

        def ln_apply(ps_h4, n, b4, g, resid, dst):
            """dst <- LayerNorm(ps_h4 + b4) * g (+ resid)."""
            nt = n // P
            h4b = hwork.tile([P, 512], bf16, tag="h4b")
            nc.scalar.activation(out=h4b[:, :n], in_=ps_h4[:, :n],
                                 func=AF.Identity, bias=b4[:])
            em = emps.tile([P, 512], bf16, tag="em")
            for i in range(nt):
                nc.tensor.transpose(out=em[:, i * P:(i + 1) * P],
                                    in_=h4b[:, i * P:(i + 1) * P],
                                    identity=ident_b[:])
            mv = stat.tile([P, 4, 2], f32, tag="mv")
            for i in range(nt):
                st = stat.tile([P, 6], f32, tag="st")
                nc.vector.bn_stats(out=st[:], in_=em[:, i * P:(i + 1) * P])
                nc.vector.bn_aggr(out=mv[:, i, :], in_=st[:])
            rstd = stat.tile([P, 4], f32, tag="rstd")
            murs = stat.tile([P, 4], f32, tag="murs")
            nc.scalar.activation(out=rstd[:, :nt], in_=mv[:, :nt, 1],
                                 func=AF.Sqrt, bias=eps_t[:])
            nc.vector.reciprocal(out=rstd[:, :nt], in_=rstd[:, :nt])
            nc.vector.tensor_tensor(out=murs[:, :nt], in0=mv[:, :nt, 0],
                                    in1=rstd[:, :nt], op=ALU.mult)
            tem = hwork.tile([P, 512], bf16, tag="tem")
            for i in range(nt):
                nc.vector.tensor_scalar(
                    out=tem[:, i * P:(i + 1) * P], in0=em[:, i * P:(i + 1) * P],
                    scalar1=rstd[:, i:i + 1], scalar2=murs[:, i:i + 1],
                    op0=ALU.mult, op1=ALU.subtract)
            tfm = mmps.tile([P, 512], bf16, tag="mm")
            for i in range(nt):
                nc.tensor.transpose(out=tfm[:, i * P:(i + 1) * P],
                                    in_=tem[:, i * P:(i + 1) * P],
                                    identity=ident_b[:])
            if resid is None:
                nc.vector.tensor_scalar(out=dst, in0=tfm[:, :n],
                                        scalar1=g[:], scalar2=None,
                                        op0=ALU.mult, op1=ALU.bypass)
            else:
                nc.vector.scalar_tensor_tensor(
                    out=dst, in0=tfm[:, :n], scalar=g[:], in1=resid,
                    op0=ALU.mult, op1=ALU.add)

        def load_inp(dram, c, n, kdim, dt=f32r):
            t = inp.tile([kdim, 512], dt, tag="inany", name="inany")
            nc.sync.dma_start(out=t[:, :n], in_=dram[:, c:c + n])
            return t

        # ---- global encoder -------------------------------------------
        partial = smps.tile([P, P], f32, tag="sm")
        for gi, (c, n) in enumerate(ngrp):
            na_t = load_inp(d_na, c, n, NODE_IN)
            ps = mm_chain(n, [(WT["glob_w0"][:], na_t[:, :n])],
                          [WT[f"glob_w{i}"] for i in range(1, 4)],
                          [VT[f"glob_b{i}"] for i in range(3)])
            xg = hwork.tile([P, 512], f32r, tag="h4b")
            nc.scalar.activation(out=xg[:, :n], in_=ps[:, :n],
                                 func=AF.Identity, bias=VT["glob_b3"][:])
            for i in range(n // P):
                b = (c // P) + i
                em = emps.tile([P, 512], f32r, tag="em")
                nc.tensor.transpose(out=em[:, :P], in_=xg[:, i * P:(i + 1) * P],
                                    identity=ident_r[:])
                emsb = hwork.tile([P, P], f32r, tag="emsb")
                nc.vector.tensor_copy(out=emsb[:], in_=em[:, :P])
                nc.tensor.matmul(out=partial[:, :NG], lhsT=emsb[:],
                                 rhs=oem_sb[:, b, :],
                                 start=(b == 0), stop=(b == NB - 1))
        partial_sb = stat.tile([P, NG], f32, tag="psb")
        nc.vector.tensor_copy(out=partial_sb[:], in_=partial[:, :NG])
        nc.sync.dma_start(out=cc_in[:], in_=partial_sb[:])
        nc.gpsimd.collective_compute(
            "AllReduce", ALU.add, ins=[cc_in[:]], outs=[cc_out[:]],
            replica_groups=[list(range(NC))])
        pooled = stat.tile([P, NG], f32r, tag="pool")
        nc.sync.dma_start(out=pooled[:], in_=cc_out[:].bitcast(f32r))
        pooledT_ps = smps.tile([P, P], f32r, tag="sm")
        nc.tensor.transpose(out=pooledT_ps[:NG, :], in_=pooled[:],
                            identity=ident_r[:])
        pooledT = stat.tile([NG, P], f32r, tag="poolT")
        nc.vector.tensor_scalar(out=pooledT[:], in0=pooledT_ps[:NG, :],
                                scalar1=ic_sb[:], scalar2=None,
                                op0=ALU.mult, op1=ALU.bypass)

        # ---- edge encoder ---------------------------------------------
        for gi, (c, n) in enumerate(egrp):
            ea_t = load_inp(d_ea, c, n, EDGE_IN, bf16)
            ps = mm_chain(n, [(WTB["eenc_w0"][:], ea_t[:, :n])],
                          [WTB[f"eenc_w{i}"] for i in range(1, 4)],
                          [VT[f"eenc_b{i}"] for i in range(3)], hdt=bf16)
            ln_apply(ps, n, VT["eenc_b3"], VT["eenc_g"], None, e_sb[gi][:, :n])

        # ---- node encoder ---------------------------------------------
        for gi, (c, n) in enumerate(ngrp):
            ofm_t = load_inp(d_ofm, c, n, NG)
            gf_ps = mmps.tile([P, 512], f32, tag="mm")
            nc.tensor.matmul(out=gf_ps[:, :n], lhsT=pooledT[:],
                             rhs=ofm_t[:, :n], start=True, stop=True)
            gf = hwork.tile([P, 512], f32r, tag="h4b", name="gf")
            nc.vector.tensor_copy(out=gf[:, :n], in_=gf_ps[:, :n])
            na_t = load_inp(d_na, c, n, NODE_IN)
            ps = mm_chain(
                n,
                [(WT["nenc_wna"][:], na_t[:, :n]),
                 (WT["nenc_wgf"][:], gf[:, :n])],
                [WT[f"nenc_w{i}"] for i in range(1, 4)],
                [VT[f"nenc_b{i}"] for i in range(3)])
            ln_apply(ps, n, VT["nenc_b3"], VT["nenc_g"], None, x_sb[gi][:, :n])

        # ---- message passing layers -----------------------------------
        for li in range(L):
            eB = [VT[f"l{li}_eb{i}"] for i in range(4)]
            eW = [WTB[f"l{li}_ew{i}"] for i in range(1, 4)]
            for gi, (c, n) in enumerate(egrp):
                ps = mm_chain(n, [(WB1[li][:], e_sb[gi][:, :n])],
                              eW, eB[:3], hdt=bf16)
                ln_apply(ps, n, eB[3], VT[f"l{li}_eg"],
                         e_sb[gi][:, :n], e_sb[gi][:, :n])
            for b in range(NB):
                s_t = spool.tile([P, TB, P], bf16, tag="S")
                nc.sync.dma_start(
                    out=s_t[:],
                    in_=d_S[b * TB:(b + 1) * TB].rearrange("t p n -> p t n"))
                aps = smps.tile([P, P], f32, tag="sm")
                for j in range(TB):
                    t = b * TB + j
                    gi, off = (t * P) // 512, (t * P) % 512
                    em = emps.tile([P, 512], bf16, tag="em")
                    nc.tensor.transpose(out=em[:, :P],
                                        in_=e_sb[gi][:, off:off + P],
                                        identity=ident_b[:])
                    emsb = hwork.tile([P, P], bf16, tag="emsb")
                    nc.vector.tensor_copy(out=emsb[:], in_=em[:, :P])
                    nc.tensor.matmul(out=aps[:], lhsT=emsb[:],
                                     rhs=s_t[:, j, :],
                                     start=(j == 0), stop=(j == TB - 1))
                gi, off = (b * P) // 512, (b * P) % 512
                nc.vector.tensor_copy(out=a_sb[gi][:, off:off + P], in_=aps[:])
            nB = [VT[f"l{li}_nb{i}"] for i in range(4)]
            for gi, (c, n) in enumerate(ngrp):
                d0_t = load_inp(d_d0, c, n, 1)
                ps = mm_chain(
                    n,
                    [(WT[f"l{li}_wx"][:], x_sb[gi][:, :n]),
                     (WT[f"l{li}_wa"][:], a_sb[gi][:, :n]),
                     (WT[f"l{li}_corr"][:], d0_t[:1, :n])],
                    [WT[f"l{li}_nw{i}"] for i in range(1, 4)],
                    nB[:3])
                ln_apply(ps, n, nB[3], VT[f"l{li}_ng"],
                         x_sb[gi][:, :n], x_sb[gi][:, :n])

        # ---- decoder ---------------------------------------------------
        dWt = [WT[f"dec_w{i}"] for i in range(3)]
        dBt = [VT[f"dec_b{i}"] for i in range(3)]
        for gi, (c, n) in enumerate(ngrp):
            ps = mm_chain(n, [(dWt[0][:], x_sb[gi][:, :n])], [dWt[1]], [dBt[0]])
            h2 = hwork.tile([P, 512], f32r, tag="h")
            nc.scalar.activation(out=h2[:, :n], in_=ps[:, :n],
                                 func=AF.Relu, bias=dBt[1][:])
            ps3 = mmps.tile([P, 512], f32, tag="mm")
            nc.tensor.matmul(out=ps3[:OUT, :n], lhsT=dWt[2][:, :OUT],
                             rhs=h2[:, :n], start=True, stop=True)
            ob = hwork.tile([OUT, 512], f32, tag="h4b", name="ob")
            nc.scalar.activation(out=ob[:, :n], in_=ps3[:OUT, :n],
                                 func=AF.Identity, bias=dBt[2][:OUT, :])
            nc.sync.dma_start(out=d_out[:, c:c + n], in_=ob[:, :n])

    split_excess_waits(nc, maxw=1)
    return nc


# ---------------------------------------------------------------------------
# entry point
# ---------------------------------------------------------------------------

_CACHE = {}


def _ensure_ntff_hook():
    """The image's antenv lacks axon_hooks; recreate it and register the
    NTFF profile hook the same way trn_agent_boot.boot() would."""
    import types
    try:
        from antenv.axon_hooks import get_axon_ntff_profile_hook  # noqa: F401
        return
    except ImportError:
        pass
    try:
        import antenv
        from trn_agent_boot.trn_boot import _ntff_profile_via_ctypes
        mod = types.ModuleType("antenv.axon_hooks")
        _h = [None]
        mod.set_axon_ntff_profile_hook = lambda h: _h.__setitem__(0, h)
        mod.get_axon_ntff_profile_hook = lambda: _h[0]
        sys.modules["antenv.axon_hooks"] = mod
        antenv.axon_hooks = mod
        mod.set_axon_ntff_profile_hook(
            _ntff_profile_via_ctypes("/opt/axon/libaxon_pjrt.so"))
    except Exception as e:  # profiling is best-effort
        print(f"ntff hook setup failed: {e}", file=sys.stderr)


def _run(inputs, trace=False):
    pp = prep(**inputs)
    wf, vf, wb, wbf = pack_blobs(pp)
    key = (pp["NPAD"], pp["EPC"], pp["TB"], wf.data().shape[0], vf.data().shape[0])
    if key not in _CACHE:
        _CACHE[key] = build(pp, wf, vf, wbf)
    nc = _CACHE[key]

    wfd, vfd = wf.data(), vf.data()
    wbfd = wbf.data().astype(BF)
    in_maps = []
    for c in range(NC):
        in_maps.append({
            "wf": wfd, "vf": vfd, "wb": wb, "wbf": wbfd,
            "eattr": pp["eattr"][c].astype(BF), "nattr": pp["nattrT"][c],
            "S": pp["S"][c], "oh_fm": pp["oh_fm"][c], "oh_em": pp["oh_em"][c],
            "deg0": pp["deg0"][c], "invc": pp["invc"],
        })
    if trace:
        _ensure_ntff_hook()
    last = None
    for attempt in range(3):
        try:
            res = run_bass_kernel_spmd(nc, in_maps, core_ids=list(range(NC)),
                                       trace=trace)
            break
        except Exception as exc:  # transient NRT device faults
            last = exc
            import time as _t
            _t.sleep(5)
    else:
        raise last
    npc = pp["npc"]
    out = np.concatenate(
        [res.results[c]["out"][:, :npc].T for c in range(NC)], 0)
    return np.ascontiguousarray(out, dtype=np.float32), res


def kernel(**inputs):
    out, _ = _run(inputs, trace=False)
    return out
